# revision 5
# baseline (speedup 1.0000x reference)
"""Multi-head attention (B=8, N=1024, C=768, H=12, D=64) on 8 TRN2 NeuronCores.

Strategy: data-parallel over batch (B == n_cores == 8), no collectives.
v2 design, fully transposed layout (channels on SBUF partitions):

  - Scores: heads processed in pairs (h0=2p on partitions 0:64, h1=2p+1 on
    64:128).  The two heads' score matmuls (contraction K=D=64) are emitted
    back-to-back with row tile_positions (0,0)/(64,0), so the PE runs them
    concurrently in separate array halves -> ~2x score throughput.
  - Score PSUM tiles are [128, 1024] (two banks: key-tiles 2jp, 2jp+1), so a
    single ACT exp instruction covers 1024 elements, amortizing overhead.
  - Bias handled as P = exp(0.125*S^T) * exp(bias)^T: exp(bias) precomputed
    host-side (bf16), applied on DVE in 2x-rate 16-bit mode (cheaper than the
    f32 PSUM add at 1x).
  - PV keeps the ones-row trick (lhsT = [V_h | 1], M=65) for softmax sums.
  - Normalization per (pair, nb-half): DVE reciprocal of the PSUM sum row,
    DMA-broadcast across partitions, fused into the PV evacuation multiply.
  - Loop order: nb (query half) outer, head-pair inner.  Bias DMA is split
    per (head, nb) so each byte is loaded once.  Output projection for
    q-half 0 runs as PE filler during q-half 1's attention.
  - QKV projections + output projection are emitted as "filler" chunks
    inside the ACT-bound attention loop to keep the PE dense (HAM warm).
"""

import os
import sys
import numpy as np

for _p in ("/opt/trn_rl_repo", "/root/.axon_site/_ro/trn_rl_repo"):
    if os.path.isdir(_p) and _p not in sys.path:
        sys.path.append(_p)

import ml_dtypes

BF16 = ml_dtypes.bfloat16

B, N, C = 8, 1024, 768
H, D = 12, 64
CT = C // 128        # 6 channel tiles
NT = N // 128        # 8 key tiles
F = 512
NP = H // 2          # 6 head pairs

_cache = {}


def _build():
    import concourse.bass as bass
    import concourse.tile as tile
    from concourse import bacc, mybir

    f32 = mybir.dt.float32
    bf16 = mybir.dt.bfloat16
    AF = mybir.ActivationFunctionType
    ALU = mybir.AluOpType

    nc = bacc.Bacc("TRN2", target_bir_lowering=False)

    xT_d = nc.dram_tensor("xT", [C, N], bf16, kind="ExternalInput")
    wqT_d = nc.dram_tensor("wqT", [C, C], bf16, kind="ExternalInput")
    wkT_d = nc.dram_tensor("wkT", [C, C], bf16, kind="ExternalInput")
    wvT_d = nc.dram_tensor("wvT", [C, C], bf16, kind="ExternalInput")
    wpT_d = nc.dram_tensor("wpT", [C, C], bf16, kind="ExternalInput")
    bpT_d = nc.dram_tensor("bpT", [128, CT], f32, kind="ExternalInput")
    expbT_d = nc.dram_tensor("expbT", [H, N, N], bf16, kind="ExternalInput")
    outT_d = nc.dram_tensor("outT", [C, N], f32, kind="ExternalOutput")
    rscr_d = nc.dram_tensor("rscr", [H, 2, F], f32)  # reciprocal bounce

    with tile.TileContext(nc) as tc:
        with tc.tile_pool(name="persist", bufs=1) as pers:
            xTb = pers.tile([128, CT, N], bf16, tag="xT")
            wqb = pers.tile([128, CT, C], bf16, tag="wq")
            wkb = pers.tile([128, CT, C], bf16, tag="wk")
            wvb = pers.tile([128, CT, C], bf16, tag="wv")
            wpb = pers.tile([128, CT, C], bf16, tag="wp")
            bpb = pers.tile([128, CT], f32, tag="bp")
            qtb = pers.tile([128, CT, N], bf16, tag="qt")
            ktb = pers.tile([128, CT, N], bf16, tag="kt")
            vb = pers.tile([128, NT, H, D + 1], bf16, tag="v")
            atb = pers.tile([128, CT, N], bf16, tag="at")
            dum = pers.tile([1, 8], f32, tag="dum")

            # ---- phase 0: input DMAs (pair-0 essentials first) -----------
            for i, q0 in enumerate(range(0, N, 256)):
                eng = nc.sync if i % 2 == 0 else nc.scalar
                eng.dma_start(
                    xTb[:, :, q0:q0 + 256],
                    xT_d[:, q0:q0 + 256].rearrange("(ci p) n -> p ci n", p=128))
            nc.scalar.dma_start(
                wvb[:, :, 0:128],
                wvT_d[:, 0:128].rearrange("(ci p) o -> p ci o", p=128))
            nc.sync.dma_start(
                wkb[:, :, 0:128],
                wkT_d[:, 0:128].rearrange("(ci p) o -> p ci o", p=128))
            nc.scalar.dma_start(
                wqb[:, :, 0:128],
                wqT_d[:, 0:128].rearrange("(ci p) o -> p ci o", p=128))
            nc.scalar.dma_start(bpb, bpT_d[:])
            nc.sync.dma_start(
                wvb[:, :, 128:C],
                wvT_d[:, 128:C].rearrange("(ci p) o -> p ci o", p=128))
            nc.scalar.dma_start(
                wkb[:, :, 128:C],
                wkT_d[:, 128:C].rearrange("(ci p) o -> p ci o", p=128))
            nc.sync.dma_start(
                wqb[:, :, 128:C],
                wqT_d[:, 128:C].rearrange("(ci p) o -> p ci o", p=128))
            nc.scalar.dma_start(
                wpb[:, :, 0:384],
                wpT_d[:, 0:384].rearrange("(ci p) o -> p ci o", p=128))
            nc.sync.dma_start(
                wpb[:, :, 384:C],
                wpT_d[:, 384:C].rearrange("(ci p) o -> p ci o", p=128))

            nc.vector.memset(vb[:, :, :, D:D + 1], 1.0)
            nc.vector.memset(dum, 1.0)
            nc.scalar.activation(dum, dum, AF.Exp)  # preload exp table set

            with tc.tile_pool(name="sps", bufs=2, space="PSUM") as sp, \
                 tc.tile_pool(name="pvps", bufs=2, space="PSUM") as pvp, \
                 tc.tile_pool(name="ups", bufs=2, space="PSUM") as ups, \
                 tc.tile_pool(name="ptp", bufs=4) as ptp, \
                 tc.tile_pool(name="btp", bufs=4) as btp, \
                 tc.tile_pool(name="rbp", bufs=4) as rbp, \
                 tc.tile_pool(name="rrp", bufs=4) as rrp, \
                 tc.tile_pool(name="vstp", bufs=2) as vstp, \
                 tc.tile_pool(name="otp", bufs=3) as otp:

                cp_state = [0]

                def cp(dst, src):
                    """PSUM->SBUF copy, alternating DVE / ACT."""
                    cp_state[0] ^= 1
                    if cp_state[0]:
                        nc.vector.tensor_copy(dst, src)
                    else:
                        nc.scalar.copy(dst, src)

                def v_chunk(p, nts):
                    """V projection for heads 2p, 2p+1 over key tiles nts."""
                    f0 = p * 128

                    def go():
                        for nt in nts:
                            ps = ups.tile([128, F], f32, tag="u")
                            for ci in range(CT):
                                nc.tensor.matmul(
                                    ps[:, 0:128],
                                    lhsT=xTb[:, ci, nt * 128:(nt + 1) * 128],
                                    rhs=wvb[:, ci, f0:f0 + 128],
                                    start=(ci == 0),
                                    stop=(ci == CT - 1),
                                )
                            cp(vb[:, nt, 2 * p:2 * p + 2, 0:D],
                               ps[:, 0:128].rearrange("p (h d) -> p h d", d=D))
                    return go

                def kq_chunk(which, cot, nbq):
                    wb, dst = (wkb, ktb) if which == "k" else (wqb, qtb)

                    def go():
                        ps = ups.tile([128, F], f32, tag="u")
                        for ci in range(CT):
                            nc.tensor.matmul(
                                ps,
                                lhsT=wb[:, ci, cot * 128:(cot + 1) * 128],
                                rhs=xTb[:, ci, nbq * F:(nbq + 1) * F],
                                start=(ci == 0),
                                stop=(ci == CT - 1),
                            )
                        cp(dst[:, cot, nbq * F:(nbq + 1) * F], ps)
                    return go

                def proj_chunk(cot, nbq, tail=False):
                    def go():
                        ps = ups.tile([128, F], f32, tag="u")
                        for ci in range(CT):
                            nc.tensor.matmul(
                                ps,
                                lhsT=wpb[:, ci, cot * 128:(cot + 1) * 128],
                                rhs=atb[:, ci, nbq * F:(nbq + 1) * F],
                                start=(ci == 0),
                                stop=(ci == CT - 1),
                            )
                        ot = otp.tile([128, F], f32, tag="ot")
                        if tail:
                            nc.scalar.activation(
                                ot, ps, AF.Identity, bias=bpb[:, cot:cot + 1])
                        else:
                            nc.vector.tensor_scalar_add(
                                ot, ps, bpb[:, cot:cot + 1])
                        eng = nc.sync if cot % 2 else nc.scalar
                        eng.dma_start(
                            outT_d[cot * 128:(cot + 1) * 128,
                                   nbq * F:(nbq + 1) * F],
                            ot,
                        )
                    return go

                fill_q = []

                def fill(n):
                    for _ in range(n):
                        if fill_q:
                            fill_q.pop(0)()

                bt_map = {}

                def bias_load(p, nbq):
                    for h in (2 * p, 2 * p + 1):
                        bt = btp.tile([128, NT, F], bf16, tag="bt")
                        src = expbT_d[h].rearrange("(j p) q -> p j q", p=128)
                        nc.sync.dma_start(
                            bt[:, 0:NT // 2, :],
                            src[:, 0:NT // 2, nbq * F:(nbq + 1) * F])
                        nc.sync.dma_start(
                            bt[:, NT // 2:NT, :],
                            src[:, NT // 2:NT, nbq * F:(nbq + 1) * F])
                        bt_map[(h, nbq)] = bt

                def attn_pair(p, nbq, nfill):
                    h0, h1 = 2 * p, 2 * p + 1
                    qsl = slice(nbq * F, (nbq + 1) * F)
                    bt0 = bt_map.pop((h0, nbq))
                    bt1 = bt_map.pop((h1, nbq))
                    pv0 = pvp.tile([D + 1, F], f32, tag="pv")
                    pv1 = pvp.tile([D + 1, F], f32, tag="pv")

                    def pv_emit(pt0, pt1, jp):
                        j0, j1 = 2 * jp, 2 * jp + 1
                        nc.tensor.matmul(
                            pv0, lhsT=vb[:, j0, h0, :], rhs=pt0[:, 0:F],
                            start=(jp == 0), stop=False)
                        nc.tensor.matmul(
                            pv1, lhsT=vb[:, j0, h1, :], rhs=pt1[:, 0:F],
                            start=(jp == 0), stop=False)
                        nc.tensor.matmul(
                            pv0, lhsT=vb[:, j1, h0, :], rhs=pt0[:, F:2 * F],
                            start=False, stop=(jp == 3))
                        nc.tensor.matmul(
                            pv1, lhsT=vb[:, j1, h1, :], rhs=pt1[:, F:2 * F],
                            start=False, stop=(jp == 3))

                    prev = None
                    for jp in range(4):
                        fill(nfill)
                        j0, j1 = 2 * jp, 2 * jp + 1
                        s0 = sp.tile([128, 2 * F], f32, tag="s")
                        s1 = sp.tile([128, 2 * F], f32, tag="s")
                        # row-tile pairs: (0,0) and (64,0) run concurrently
                        nc.tensor.matmul(
                            s0[:, 0:F],
                            lhsT=ktb[0:64, p, j0 * 128:(j0 + 1) * 128],
                            rhs=qtb[0:64, p, qsl], start=True, stop=True)
                        nc.tensor.matmul(
                            s1[:, 0:F],
                            lhsT=ktb[64:128, p, j0 * 128:(j0 + 1) * 128],
                            rhs=qtb[64:128, p, qsl], start=True, stop=True)
                        nc.tensor.matmul(
                            s0[:, F:2 * F],
                            lhsT=ktb[0:64, p, j1 * 128:(j1 + 1) * 128],
                            rhs=qtb[0:64, p, qsl], start=True, stop=True)
                        nc.tensor.matmul(
                            s1[:, F:2 * F],
                            lhsT=ktb[64:128, p, j1 * 128:(j1 + 1) * 128],
                            rhs=qtb[64:128, p, qsl], start=True, stop=True)
                        pt0 = ptp.tile([128, 2 * F], bf16, tag="pt")
                        pt1 = ptp.tile([128, 2 * F], bf16, tag="pt")
                        nc.scalar.activation(pt0, s0, AF.Exp, scale=0.125)
                        nc.vector.tensor_tensor(
                            pt0.rearrange("p (j q) -> p j q", j=2),
                            pt0.rearrange("p (j q) -> p j q", j=2),
                            bt0[:, j0:j0 + 2, :], ALU.mult)
                        nc.scalar.activation(pt1, s1, AF.Exp, scale=0.125)
                        nc.vector.tensor_tensor(
                            pt1.rearrange("p (j q) -> p j q", j=2),
                            pt1.rearrange("p (j q) -> p j q", j=2),
                            bt1[:, j0:j0 + 2, :], ALU.mult)
                        if prev is not None:
                            pv_emit(*prev)
                        prev = (pt0, pt1, jp)
                    pv_emit(*prev)

                    # ---- pair tail: reciprocal + fused normalize/evac ----
                    rr0 = rrp.tile([1, F], f32, tag="rr")
                    nc.vector.reciprocal(rr0, pv0[D:D + 1, :])
                    nc.gpsimd.dma_start(rscr_d[h0, nbq:nbq + 1, :], rr0)
                    rb0 = rbp.tile([128, F], f32, tag="rb")
                    nc.sync.dma_start(
                        rb0, rscr_d[h0, nbq:nbq + 1, :].to_broadcast([128, F]))
                    nc.vector.tensor_tensor(
                        atb[0:64, p, qsl], pv0[0:D, :], rb0[0:64, :], ALU.mult)
                    rr1 = rrp.tile([1, F], f32, tag="rr")
                    nc.vector.reciprocal(rr1, pv1[D:D + 1, :])
                    nc.gpsimd.dma_start(rscr_d[h1, nbq:nbq + 1, :], rr1)
                    rb1 = rbp.tile([128, F], f32, tag="rb")
                    nc.sync.dma_start(
                        rb1, rscr_d[h1, nbq:nbq + 1, :].to_broadcast([128, F]))
                    vst = vstp.tile([D, F], bf16, tag="vst")
                    nc.vector.tensor_tensor(
                        vst, pv1[0:D, :], rb1[0:64, :], ALU.mult)
                    nc.gpsimd.dma_start(atb[64:128, p, qsl], vst)

                # ---- emission schedule --------------------------------
                bias_load(0, 0)
                # pair-0 essentials, emitted directly
                v_chunk(0, range(0, NT))()
                kq_chunk("k", 0, 0)()
                kq_chunk("k", 0, 1)()
                kq_chunk("q", 0, 0)()

                # filler for nb0 pass: V/K/Q for pairs 1..5, then Q nb1
                for p in range(1, NP):
                    fill_q.append(v_chunk(p, range(0, NT // 2)))
                    fill_q.append(v_chunk(p, range(NT // 2, NT)))
                    fill_q.append(kq_chunk("k", p, 0))
                    fill_q.append(kq_chunk("k", p, 1))
                    fill_q.append(kq_chunk("q", p, 0))
                for p in range(NP):
                    fill_q.append(kq_chunk("q", p, 1))

                seq = [(p, 0) for p in range(NP)] + [(p, 1) for p in range(NP)]
                for i, (p, nbq) in enumerate(seq):
                    if i + 1 < len(seq):
                        bias_load(*seq[i + 1])
                    if nbq == 1 and p == 0:
                        # proj for q-half 0 becomes filler during nb1 pass
                        for cot in range(CT):
                            fill_q.append(proj_chunk(cot, 0))
                    attn_pair(p, nbq, nfill=2 if nbq == 0 else 1)
                fill(len(fill_q))
                # tail: projection for q-half 1
                for cot in range(CT):
                    proj_chunk(cot, 1, tail=True)()

    nc.compile()
    return nc


def _get_nc():
    if "nc" not in _cache:
        _cache["nc"] = _build()
    return _cache["nc"]


def prep_in_maps(x, attn_bias, Wq, Wk, Wv, Wp, bp):
    """Host-side sharding + layout prep (transposes/casts only)."""
    wqT = np.ascontiguousarray(Wq.T).astype(BF16)
    wkT = np.ascontiguousarray(Wk.T).astype(BF16)
    wvT = np.ascontiguousarray(Wv.T).astype(BF16)
    wpT = np.ascontiguousarray(Wp.T).astype(BF16)
    bpT = np.ascontiguousarray(bp.astype(np.float32).reshape(CT, 128).T)
    expbT = np.ascontiguousarray(
        np.exp(attn_bias[0].astype(np.float32)).transpose(0, 2, 1)
    ).astype(BF16)
    in_maps = []
    for b in range(B):
        in_maps.append({
            "xT": np.ascontiguousarray(x[b].T).astype(BF16),
            "wqT": wqT, "wkT": wkT, "wvT": wvT, "wpT": wpT,
            "bpT": bpT, "expbT": expbT,
        })
    return in_maps


def run(in_maps, trace=False, **kw):
    from concourse.bass_utils import run_bass_kernel_spmd

    nc = _get_nc()
    return run_bass_kernel_spmd(
        nc, in_maps, core_ids=list(range(B)), trace=trace, **kw
    )


def kernel(x, attn_bias, Wq, Wk, Wv, Wp, bp):
    res = run(prep_in_maps(x, attn_bias, Wq, Wk, Wv, Wp, bp))
    out = np.stack(
        [res.results[b]["outT"].T for b in range(B)]
    ).astype(np.float32)
    return out


# revision 7
# speedup vs baseline: 1.0906x; 1.0906x over previous
"""Multi-head attention (B=8, N=1024, C=768, H=12, D=64) on 8 TRN2 NeuronCores.

Strategy: data-parallel over batch (B == n_cores == 8), no collectives.
v2 design, fully transposed layout (channels on SBUF partitions):

  - Scores: heads processed in pairs (h0=2p on partitions 0:64, h1=2p+1 on
    64:128).  The two heads' score matmuls (contraction K=D=64) are emitted
    back-to-back with row tile_positions (0,0)/(64,0), so the PE runs them
    concurrently in separate array halves -> ~2x score throughput.
  - Score PSUM tiles are [128, 1024] (two banks: key-tiles 2jp, 2jp+1), so a
    single ACT exp instruction covers 1024 elements, amortizing overhead.
  - Bias handled as P = exp(0.125*S^T) * exp(bias)^T: exp(bias) precomputed
    host-side (bf16), applied on DVE in 2x-rate 16-bit mode (cheaper than the
    f32 PSUM add at 1x).
  - PV keeps the ones-row trick (lhsT = [V_h | 1], M=65) for softmax sums.
  - Normalization per (pair, nb-half): DVE reciprocal of the PSUM sum row,
    DMA-broadcast across partitions, fused into the PV evacuation multiply.
  - Loop order: nb (query half) outer, head-pair inner.  Bias DMA is split
    per (head, nb) so each byte is loaded once.  Output projection for
    q-half 0 runs as PE filler during q-half 1's attention.
  - QKV projections + output projection are emitted as "filler" chunks
    inside the ACT-bound attention loop to keep the PE dense (HAM warm).
"""

import os
import sys
import numpy as np

for _p in ("/opt/trn_rl_repo", "/root/.axon_site/_ro/trn_rl_repo"):
    if os.path.isdir(_p) and _p not in sys.path:
        sys.path.append(_p)

import ml_dtypes

BF16 = ml_dtypes.bfloat16

B, N, C = 8, 1024, 768
H, D = 12, 64
CT = C // 128        # 6 channel tiles
NT = N // 128        # 8 key tiles
F = 512
NP = H // 2          # 6 head pairs

_cache = {}


def _build():
    import concourse.bass as bass
    import concourse.tile as tile
    from concourse import bacc, mybir

    f32 = mybir.dt.float32
    bf16 = mybir.dt.bfloat16
    AF = mybir.ActivationFunctionType
    ALU = mybir.AluOpType

    nc = bacc.Bacc("TRN2", target_bir_lowering=False)

    xT_d = nc.dram_tensor("xT", [C, N], bf16, kind="ExternalInput")
    wqT_d = nc.dram_tensor("wqT", [C, C], bf16, kind="ExternalInput")
    wkT_d = nc.dram_tensor("wkT", [C, C], bf16, kind="ExternalInput")
    wvT_d = nc.dram_tensor("wvT", [C, C], bf16, kind="ExternalInput")
    wpT_d = nc.dram_tensor("wpT", [C, C], bf16, kind="ExternalInput")
    bpT_d = nc.dram_tensor("bpT", [128, CT], f32, kind="ExternalInput")
    expbT_d = nc.dram_tensor("expbT", [H, N, N], bf16, kind="ExternalInput")
    outT_d = nc.dram_tensor("outT", [C, N], f32, kind="ExternalOutput")
    rscr_d = nc.dram_tensor("rscr", [H, 2, F], f32)  # reciprocal bounce

    with tile.TileContext(nc) as tc:
        with tc.tile_pool(name="persist", bufs=1) as pers:
            xTb = pers.tile([128, CT, N], bf16, tag="xT")
            wqb = pers.tile([128, CT, C], bf16, tag="wq")
            wkb = pers.tile([128, CT, C], bf16, tag="wk")
            wvb = pers.tile([128, CT, C], bf16, tag="wv")
            wpb = pers.tile([128, CT, C], bf16, tag="wp")
            bpb = pers.tile([128, CT], f32, tag="bp")
            qtb = pers.tile([128, CT, N], bf16, tag="qt")
            ktb = pers.tile([128, CT, N], bf16, tag="kt")
            vb = pers.tile([128, NT, H, D + 1], bf16, tag="v")
            atb = pers.tile([128, CT, N], bf16, tag="at")
            dum = pers.tile([1, 8], f32, tag="dum")

            # ---- phase 0: input DMAs (pair-0 essentials first) -----------
            for i, q0 in enumerate(range(0, N, 256)):
                eng = nc.sync if i % 2 == 0 else nc.scalar
                eng.dma_start(
                    xTb[:, :, q0:q0 + 256],
                    xT_d[:, q0:q0 + 256].rearrange("(ci p) n -> p ci n", p=128))
            nc.scalar.dma_start(
                wvb[:, :, 0:128],
                wvT_d[:, 0:128].rearrange("(ci p) o -> p ci o", p=128))
            nc.sync.dma_start(
                wkb[:, :, 0:128],
                wkT_d[:, 0:128].rearrange("(ci p) o -> p ci o", p=128))
            nc.scalar.dma_start(
                wqb[:, :, 0:128],
                wqT_d[:, 0:128].rearrange("(ci p) o -> p ci o", p=128))
            nc.scalar.dma_start(bpb, bpT_d[:])
            nc.sync.dma_start(
                wvb[:, :, 128:C],
                wvT_d[:, 128:C].rearrange("(ci p) o -> p ci o", p=128))
            nc.scalar.dma_start(
                wkb[:, :, 128:C],
                wkT_d[:, 128:C].rearrange("(ci p) o -> p ci o", p=128))
            nc.sync.dma_start(
                wqb[:, :, 128:C],
                wqT_d[:, 128:C].rearrange("(ci p) o -> p ci o", p=128))
            nc.scalar.dma_start(
                wpb[:, :, 0:384],
                wpT_d[:, 0:384].rearrange("(ci p) o -> p ci o", p=128))
            nc.sync.dma_start(
                wpb[:, :, 384:C],
                wpT_d[:, 384:C].rearrange("(ci p) o -> p ci o", p=128))

            nc.vector.memset(vb[:, :, :, D:D + 1], 1.0)
            nc.vector.memset(dum, 1.0)
            nc.scalar.activation(dum, dum, AF.Exp)  # preload exp table set

            with tc.tile_pool(name="sps", bufs=2, space="PSUM") as sp, \
                 tc.tile_pool(name="pvps", bufs=2, space="PSUM") as pvp, \
                 tc.tile_pool(name="ups", bufs=2, space="PSUM") as ups, \
                 tc.tile_pool(name="ptp", bufs=4) as ptp, \
                 tc.tile_pool(name="btp", bufs=4) as btp, \
                 tc.tile_pool(name="rbp", bufs=4) as rbp, \
                 tc.tile_pool(name="rrp", bufs=4) as rrp, \
                 tc.tile_pool(name="vstp", bufs=2) as vstp, \
                 tc.tile_pool(name="otp", bufs=3) as otp:

                cp_state = [0]

                def cp(dst, src):
                    """PSUM->SBUF copy, 2:1 DVE:ACT."""
                    cp_state[0] = (cp_state[0] + 1) % 3
                    if cp_state[0]:
                        nc.vector.tensor_copy(dst, src)
                    else:
                        nc.scalar.copy(dst, src)

                def v_chunk(p, nts):
                    """V projection for heads 2p, 2p+1 over key tiles nts."""
                    f0 = p * 128

                    def go():
                        for nt in nts:
                            ps = ups.tile([128, F], f32, tag="u")
                            for ci in range(CT):
                                nc.tensor.matmul(
                                    ps[:, 0:128],
                                    lhsT=xTb[:, ci, nt * 128:(nt + 1) * 128],
                                    rhs=wvb[:, ci, f0:f0 + 128],
                                    start=(ci == 0),
                                    stop=(ci == CT - 1),
                                )
                            cp(vb[:, nt, 2 * p:2 * p + 2, 0:D],
                               ps[:, 0:128].rearrange("p (h d) -> p h d", d=D))
                    return go

                def kq_chunk(which, cot, nbq):
                    wb, dst = (wkb, ktb) if which == "k" else (wqb, qtb)

                    def go():
                        ps = ups.tile([128, F], f32, tag="u")
                        for ci in range(CT):
                            nc.tensor.matmul(
                                ps,
                                lhsT=wb[:, ci, cot * 128:(cot + 1) * 128],
                                rhs=xTb[:, ci, nbq * F:(nbq + 1) * F],
                                start=(ci == 0),
                                stop=(ci == CT - 1),
                            )
                        cp(dst[:, cot, nbq * F:(nbq + 1) * F], ps)
                    return go

                def proj_chunk(cot, nbq, tail=False):
                    def go():
                        ps = ups.tile([128, F], f32, tag="u")
                        for ci in range(CT):
                            nc.tensor.matmul(
                                ps,
                                lhsT=wpb[:, ci, cot * 128:(cot + 1) * 128],
                                rhs=atb[:, ci, nbq * F:(nbq + 1) * F],
                                start=(ci == 0),
                                stop=(ci == CT - 1),
                            )
                        ot = otp.tile([128, F], f32, tag="ot")
                        if tail:
                            nc.scalar.activation(
                                ot, ps, AF.Identity, bias=bpb[:, cot:cot + 1])
                        else:
                            nc.vector.tensor_scalar_add(
                                ot, ps, bpb[:, cot:cot + 1])
                        eng = nc.sync if cot % 2 else nc.scalar
                        eng.dma_start(
                            outT_d[cot * 128:(cot + 1) * 128,
                                   nbq * F:(nbq + 1) * F],
                            ot,
                        )
                    return go

                fill_q = []

                def fill(n):
                    for _ in range(n):
                        if fill_q:
                            fill_q.pop(0)()

                bt_map = {}

                def bias_load(p, nbq):
                    for h in (2 * p, 2 * p + 1):
                        bt = btp.tile([128, NT, F], bf16, tag="bt")
                        src = expbT_d[h].rearrange("(j p) q -> p j q", p=128)
                        nc.sync.dma_start(
                            bt[:, 0:NT // 2, :],
                            src[:, 0:NT // 2, nbq * F:(nbq + 1) * F])
                        nc.sync.dma_start(
                            bt[:, NT // 2:NT, :],
                            src[:, NT // 2:NT, nbq * F:(nbq + 1) * F])
                        bt_map[(h, nbq)] = bt

                def attn_pair(p, nbq, nfill):
                    h0, h1 = 2 * p, 2 * p + 1
                    qsl = slice(nbq * F, (nbq + 1) * F)
                    bt0 = bt_map.pop((h0, nbq))
                    bt1 = bt_map.pop((h1, nbq))
                    pv0 = pvp.tile([D + 1, F], f32, tag="pv")
                    pv1 = pvp.tile([D + 1, F], f32, tag="pv")

                    def pv_emit(pt0, pt1, jp):
                        j0, j1 = 2 * jp, 2 * jp + 1
                        nc.tensor.matmul(
                            pv0, lhsT=vb[:, j0, h0, :], rhs=pt0[:, 0:F],
                            start=(jp == 0), stop=False)
                        nc.tensor.matmul(
                            pv1, lhsT=vb[:, j0, h1, :], rhs=pt1[:, 0:F],
                            start=(jp == 0), stop=False)
                        nc.tensor.matmul(
                            pv0, lhsT=vb[:, j1, h0, :], rhs=pt0[:, F:2 * F],
                            start=False, stop=(jp == 3))
                        nc.tensor.matmul(
                            pv1, lhsT=vb[:, j1, h1, :], rhs=pt1[:, F:2 * F],
                            start=False, stop=(jp == 3))

                    prev = None
                    for jp in range(4):
                        fill(nfill)
                        j0, j1 = 2 * jp, 2 * jp + 1
                        s0 = sp.tile([128, 2 * F], f32, tag="s")
                        s1 = sp.tile([128, 2 * F], f32, tag="s")
                        # row-tile pairs: (0,0) and (64,0) run concurrently
                        nc.tensor.matmul(
                            s0[:, 0:F],
                            lhsT=ktb[0:64, p, j0 * 128:(j0 + 1) * 128],
                            rhs=qtb[0:64, p, qsl], start=True, stop=True)
                        nc.tensor.matmul(
                            s1[:, 0:F],
                            lhsT=ktb[64:128, p, j0 * 128:(j0 + 1) * 128],
                            rhs=qtb[64:128, p, qsl], start=True, stop=True)
                        nc.tensor.matmul(
                            s0[:, F:2 * F],
                            lhsT=ktb[0:64, p, j1 * 128:(j1 + 1) * 128],
                            rhs=qtb[0:64, p, qsl], start=True, stop=True)
                        nc.tensor.matmul(
                            s1[:, F:2 * F],
                            lhsT=ktb[64:128, p, j1 * 128:(j1 + 1) * 128],
                            rhs=qtb[64:128, p, qsl], start=True, stop=True)
                        pt0 = ptp.tile([128, 2 * F], bf16, tag="pt")
                        pt1 = ptp.tile([128, 2 * F], bf16, tag="pt")
                        nc.scalar.activation(pt0, s0, AF.Exp, scale=0.125)
                        nc.vector.tensor_tensor(
                            pt0.rearrange("p (j q) -> p j q", j=2),
                            pt0.rearrange("p (j q) -> p j q", j=2),
                            bt0[:, j0:j0 + 2, :], ALU.mult)
                        nc.scalar.activation(pt1, s1, AF.Exp, scale=0.125)
                        nc.vector.tensor_tensor(
                            pt1.rearrange("p (j q) -> p j q", j=2),
                            pt1.rearrange("p (j q) -> p j q", j=2),
                            bt1[:, j0:j0 + 2, :], ALU.mult)
                        if prev is not None:
                            pv_emit(*prev)
                        prev = (pt0, pt1, jp)
                    pv_emit(*prev)

                    # ---- pair tail: reciprocal + fused normalize/evac ----
                    sr0 = rrp.tile([1, F], f32, tag="sr")
                    nc.vector.tensor_copy(sr0, pv0[D:D + 1, :])
                    rr0 = rrp.tile([1, F], f32, tag="rr")
                    nc.vector.reciprocal_approx_fast(rr0, sr0)
                    nc.gpsimd.dma_start(rscr_d[h0, nbq:nbq + 1, :], rr0)
                    rb0 = rbp.tile([128, F], f32, tag="rb")
                    nc.sync.dma_start(
                        rb0, rscr_d[h0, nbq:nbq + 1, :].to_broadcast([128, F]))
                    nc.vector.tensor_tensor(
                        atb[0:64, p, qsl], pv0[0:D, :], rb0[0:64, :], ALU.mult)
                    sr1 = rrp.tile([1, F], f32, tag="sr")
                    nc.vector.tensor_copy(sr1, pv1[D:D + 1, :])
                    rr1 = rrp.tile([1, F], f32, tag="rr")
                    nc.vector.reciprocal_approx_fast(rr1, sr1)
                    nc.gpsimd.dma_start(rscr_d[h1, nbq:nbq + 1, :], rr1)
                    rb1 = rbp.tile([128, F], f32, tag="rb")
                    nc.sync.dma_start(
                        rb1, rscr_d[h1, nbq:nbq + 1, :].to_broadcast([128, F]))
                    vst = vstp.tile([D, F], bf16, tag="vst")
                    nc.vector.tensor_tensor(
                        vst, pv1[0:D, :], rb1[0:64, :], ALU.mult)
                    nc.gpsimd.dma_start(atb[64:128, p, qsl], vst)

                # ---- emission schedule --------------------------------
                bias_load(0, 0)
                # pair-0 essentials, emitted directly
                v_chunk(0, range(0, NT))()
                kq_chunk("k", 0, 0)()
                kq_chunk("k", 0, 1)()
                kq_chunk("q", 0, 0)()

                # filler for nb0 pass: V/K/Q for pairs 1..5, then Q nb1
                for p in range(1, NP):
                    fill_q.append(v_chunk(p, range(0, NT // 2)))
                    fill_q.append(v_chunk(p, range(NT // 2, NT)))
                    fill_q.append(kq_chunk("k", p, 0))
                    fill_q.append(kq_chunk("k", p, 1))
                    fill_q.append(kq_chunk("q", p, 0))
                for p in range(NP):
                    fill_q.append(kq_chunk("q", p, 1))

                seq = [(p, 0) for p in range(NP)] + [(p, 1) for p in range(NP)]
                for i, (p, nbq) in enumerate(seq):
                    if i + 1 < len(seq):
                        bias_load(*seq[i + 1])
                    if nbq == 1 and p == 0:
                        # proj for q-half 0 becomes filler during nb1 pass
                        for cot in range(CT):
                            fill_q.append(proj_chunk(cot, 0))
                    attn_pair(p, nbq, nfill=2 if nbq == 0 else 1)
                fill(len(fill_q))
                # tail: projection for q-half 1
                for cot in range(CT):
                    proj_chunk(cot, 1, tail=True)()

    nc.compile()
    return nc


def _get_nc():
    if "nc" not in _cache:
        _cache["nc"] = _build()
    return _cache["nc"]


def prep_in_maps(x, attn_bias, Wq, Wk, Wv, Wp, bp):
    """Host-side sharding + layout prep (transposes/casts only)."""
    wqT = np.ascontiguousarray(Wq.T).astype(BF16)
    wkT = np.ascontiguousarray(Wk.T).astype(BF16)
    wvT = np.ascontiguousarray(Wv.T).astype(BF16)
    wpT = np.ascontiguousarray(Wp.T).astype(BF16)
    bpT = np.ascontiguousarray(bp.astype(np.float32).reshape(CT, 128).T)
    expbT = np.ascontiguousarray(
        np.exp(attn_bias[0].astype(np.float32)).transpose(0, 2, 1)
    ).astype(BF16)
    in_maps = []
    for b in range(B):
        in_maps.append({
            "xT": np.ascontiguousarray(x[b].T).astype(BF16),
            "wqT": wqT, "wkT": wkT, "wvT": wvT, "wpT": wpT,
            "bpT": bpT, "expbT": expbT,
        })
    return in_maps


def run(in_maps, trace=False, **kw):
    from concourse.bass_utils import run_bass_kernel_spmd

    nc = _get_nc()
    return run_bass_kernel_spmd(
        nc, in_maps, core_ids=list(range(B)), trace=trace, **kw
    )


def kernel(x, attn_bias, Wq, Wk, Wv, Wp, bp):
    res = run(prep_in_maps(x, attn_bias, Wq, Wk, Wv, Wp, bp))
    out = np.stack(
        [res.results[b]["outT"].T for b in range(B)]
    ).astype(np.float32)
    return out


# revision 10
# speedup vs baseline: 1.1758x; 1.0781x over previous
"""Multi-head attention (B=8, N=1024, C=768, H=12, D=64) on 8 TRN2 NeuronCores.

Strategy: data-parallel over batch (B == n_cores == 8), no collectives.
v2 design, fully transposed layout (channels on SBUF partitions):

  - Scores: heads processed in pairs (h0=2p on partitions 0:64, h1=2p+1 on
    64:128).  The two heads' score matmuls (contraction K=D=64) are emitted
    back-to-back with row tile_positions (0,0)/(64,0), so the PE runs them
    concurrently in separate array halves -> ~2x score throughput.
  - Score PSUM tiles are [128, 1024] (two banks: key-tiles 2jp, 2jp+1), so a
    single ACT exp instruction covers 1024 elements, amortizing overhead.
  - Bias handled as P = exp(0.125*S^T) * exp(bias)^T: exp(bias) precomputed
    host-side (bf16), applied on DVE in 2x-rate 16-bit mode (cheaper than the
    f32 PSUM add at 1x).
  - PV keeps the ones-row trick (lhsT = [V_h | 1], M=65) for softmax sums.
  - Normalization per (pair, nb-half): DVE reciprocal of the PSUM sum row,
    DMA-broadcast across partitions, fused into the PV evacuation multiply.
  - Loop order: nb (query half) outer, head-pair inner.  Bias DMA is split
    per (head, nb) so each byte is loaded once.  Output projection for
    q-half 0 runs as PE filler during q-half 1's attention.
  - QKV projections + output projection are emitted as "filler" chunks
    inside the ACT-bound attention loop to keep the PE dense (HAM warm).
"""

import os
import sys
import numpy as np

for _p in ("/opt/trn_rl_repo", "/root/.axon_site/_ro/trn_rl_repo"):
    if os.path.isdir(_p) and _p not in sys.path:
        sys.path.append(_p)

import ml_dtypes

BF16 = ml_dtypes.bfloat16

B, N, C = 8, 1024, 768
H, D = 12, 64
CT = C // 128        # 6 channel tiles
NT = N // 128        # 8 key tiles
F = 512
NP = H // 2          # 6 head pairs

_cache = {}


def _build():
    import concourse.bass as bass
    import concourse.tile as tile
    from concourse import bacc, mybir

    f32 = mybir.dt.float32
    bf16 = mybir.dt.bfloat16
    AF = mybir.ActivationFunctionType
    ALU = mybir.AluOpType

    nc = bacc.Bacc("TRN2", target_bir_lowering=False)

    xT_d = nc.dram_tensor("xT", [C, N], bf16, kind="ExternalInput")
    wqT_d = nc.dram_tensor("wqT", [C, C], bf16, kind="ExternalInput")
    wkT_d = nc.dram_tensor("wkT", [C, C], bf16, kind="ExternalInput")
    wvT_d = nc.dram_tensor("wvT", [C, C], bf16, kind="ExternalInput")
    wpT_d = nc.dram_tensor("wpT", [C, C], bf16, kind="ExternalInput")
    bpT_d = nc.dram_tensor("bpT", [128, CT], f32, kind="ExternalInput")
    expbT_d = nc.dram_tensor("expbT", [H, N, N], bf16, kind="ExternalInput")
    outT_d = nc.dram_tensor("outT", [C, N], f32, kind="ExternalOutput")
    rscr_d = nc.dram_tensor("rscr", [H, 2, F], f32)  # reciprocal bounce

    with tile.TileContext(nc) as tc:
        with tc.tile_pool(name="persist", bufs=1) as pers:
            xTb = pers.tile([128, CT, N], bf16, tag="xT")
            wqb = pers.tile([128, CT, C], bf16, tag="wq")
            wkb = pers.tile([128, CT, C], bf16, tag="wk")
            wvb = pers.tile([128, CT, C], bf16, tag="wv")
            wpb = pers.tile([128, CT, C], bf16, tag="wp")
            bpb = pers.tile([128, CT], f32, tag="bp")
            qtb = pers.tile([128, CT, N], bf16, tag="qt")
            ktb = pers.tile([128, CT, N], bf16, tag="kt")
            vb = pers.tile([128, NT, H, D + 1], bf16, tag="v")
            atb = pers.tile([128, CT, N], bf16, tag="at")
            dum = pers.tile([1, 8], f32, tag="dum")

            # ---- phase 0: input DMAs (pair-0 essentials first) -----------
            for i, q0 in enumerate(range(0, N, 256)):
                eng = nc.sync if i % 2 == 0 else nc.scalar
                eng.dma_start(
                    xTb[:, :, q0:q0 + 256],
                    xT_d[:, q0:q0 + 256].rearrange("(ci p) n -> p ci n", p=128))
            nc.scalar.dma_start(
                wvb[:, :, 0:128],
                wvT_d[:, 0:128].rearrange("(ci p) o -> p ci o", p=128))
            nc.sync.dma_start(
                wkb[:, :, 0:128],
                wkT_d[:, 0:128].rearrange("(ci p) o -> p ci o", p=128))
            nc.scalar.dma_start(
                wqb[:, :, 0:128],
                wqT_d[:, 0:128].rearrange("(ci p) o -> p ci o", p=128))
            nc.scalar.dma_start(bpb, bpT_d[:])
            nc.sync.dma_start(
                wvb[:, :, 128:C],
                wvT_d[:, 128:C].rearrange("(ci p) o -> p ci o", p=128))
            nc.scalar.dma_start(
                wkb[:, :, 128:C],
                wkT_d[:, 128:C].rearrange("(ci p) o -> p ci o", p=128))
            nc.sync.dma_start(
                wqb[:, :, 128:C],
                wqT_d[:, 128:C].rearrange("(ci p) o -> p ci o", p=128))
            nc.scalar.dma_start(
                wpb[:, :, 0:384],
                wpT_d[:, 0:384].rearrange("(ci p) o -> p ci o", p=128))
            nc.sync.dma_start(
                wpb[:, :, 384:C],
                wpT_d[:, 384:C].rearrange("(ci p) o -> p ci o", p=128))

            nc.vector.memset(vb[:, :, :, D:D + 1], 1.0)
            nc.vector.memset(dum, 1.0)
            nc.scalar.activation(dum, dum, AF.Exp)  # preload exp table set

            with tc.tile_pool(name="sps", bufs=2, space="PSUM") as sp, \
                 tc.tile_pool(name="pvps", bufs=2, space="PSUM") as pvp, \
                 tc.tile_pool(name="ups", bufs=2, space="PSUM") as ups, \
                 tc.tile_pool(name="ptp", bufs=6) as ptp, \
                 tc.tile_pool(name="btp", bufs=4) as btp, \
                 tc.tile_pool(name="rbp", bufs=4) as rbp, \
                 tc.tile_pool(name="rrp", bufs=4) as rrp, \
                 tc.tile_pool(name="vstp", bufs=2) as vstp, \
                 tc.tile_pool(name="otp", bufs=3) as otp:

                cp_state = [0]

                def cp(dst, src):
                    """PSUM->SBUF copy, 2:1 DVE:ACT."""
                    cp_state[0] = (cp_state[0] + 1) % 3
                    if cp_state[0]:
                        nc.vector.tensor_copy(dst, src)
                    else:
                        nc.scalar.copy(dst, src)

                def v_chunk(p, nts):
                    """V projection for heads 2p, 2p+1 over key tiles nts."""
                    f0 = p * 128

                    def go():
                        for nt in nts:
                            ps = ups.tile([128, F], f32, tag="u")
                            for ci in range(CT):
                                nc.tensor.matmul(
                                    ps[:, 0:128],
                                    lhsT=xTb[:, ci, nt * 128:(nt + 1) * 128],
                                    rhs=wvb[:, ci, f0:f0 + 128],
                                    start=(ci == 0),
                                    stop=(ci == CT - 1),
                                )
                            cp(vb[:, nt, 2 * p:2 * p + 2, 0:D],
                               ps[:, 0:128].rearrange("p (h d) -> p h d", d=D))
                    return go

                def kq_chunk(which, cot, nbq):
                    wb, dst = (wkb, ktb) if which == "k" else (wqb, qtb)

                    def go():
                        ps = ups.tile([128, F], f32, tag="u")
                        for ci in range(CT):
                            nc.tensor.matmul(
                                ps,
                                lhsT=wb[:, ci, cot * 128:(cot + 1) * 128],
                                rhs=xTb[:, ci, nbq * F:(nbq + 1) * F],
                                start=(ci == 0),
                                stop=(ci == CT - 1),
                            )
                        cp(dst[:, cot, nbq * F:(nbq + 1) * F], ps)
                    return go

                def proj_chunk(cot, nbq, tail=False):
                    def go():
                        ps = ups.tile([128, F], f32, tag="u")
                        for ci in range(CT):
                            nc.tensor.matmul(
                                ps,
                                lhsT=wpb[:, ci, cot * 128:(cot + 1) * 128],
                                rhs=atb[:, ci, nbq * F:(nbq + 1) * F],
                                start=(ci == 0),
                                stop=(ci == CT - 1),
                            )
                        ot = otp.tile([128, F], f32, tag="ot")
                        if tail:
                            nc.scalar.activation(
                                ot, ps, AF.Identity, bias=bpb[:, cot:cot + 1])
                        else:
                            nc.vector.tensor_scalar_add(
                                ot, ps, bpb[:, cot:cot + 1])
                        eng = nc.sync if cot % 2 else nc.scalar
                        eng.dma_start(
                            outT_d[cot * 128:(cot + 1) * 128,
                                   nbq * F:(nbq + 1) * F],
                            ot,
                        )
                    return go

                fill_q = []

                def fill(n):
                    for _ in range(n):
                        if fill_q:
                            fill_q.pop(0)()

                bt_map = {}

                def bias_load(p, nbq):
                    for h in (2 * p, 2 * p + 1):
                        bt = btp.tile([128, NT, F], bf16, tag="bt")
                        src = expbT_d[h].rearrange("(j p) q -> p j q", p=128)
                        nc.sync.dma_start(
                            bt[:, 0:NT // 2, :],
                            src[:, 0:NT // 2, nbq * F:(nbq + 1) * F])
                        nc.sync.dma_start(
                            bt[:, NT // 2:NT, :],
                            src[:, NT // 2:NT, nbq * F:(nbq + 1) * F])
                        bt_map[(h, nbq)] = bt

                pair_state = {}

                def scores_emit(p, nbq, jp):
                    """Score matmuls + exp + bias-mult for one jp slot."""
                    h0, h1 = 2 * p, 2 * p + 1
                    qsl = slice(nbq * F, (nbq + 1) * F)
                    if jp == 0:
                        pair_state[(p, nbq)] = {
                            "bt0": bt_map.pop((h0, nbq)),
                            "bt1": bt_map.pop((h1, nbq)),
                        }
                    st = pair_state[(p, nbq)]
                    j0, j1 = 2 * jp, 2 * jp + 1
                    s0 = sp.tile([128, 2 * F], f32, tag="s", name=f"s0_{jp}")
                    s1 = sp.tile([128, 2 * F], f32, tag="s", name=f"s1_{jp}")
                    # row-tile pairs: (0,0) and (64,0) run concurrently
                    nc.tensor.matmul(
                        s0[:, 0:F],
                        lhsT=ktb[0:64, p, j0 * 128:(j0 + 1) * 128],
                        rhs=qtb[0:64, p, qsl], start=True, stop=True)
                    nc.tensor.matmul(
                        s1[:, 0:F],
                        lhsT=ktb[64:128, p, j0 * 128:(j0 + 1) * 128],
                        rhs=qtb[64:128, p, qsl], start=True, stop=True)
                    nc.tensor.matmul(
                        s0[:, F:2 * F],
                        lhsT=ktb[0:64, p, j1 * 128:(j1 + 1) * 128],
                        rhs=qtb[0:64, p, qsl], start=True, stop=True)
                    nc.tensor.matmul(
                        s1[:, F:2 * F],
                        lhsT=ktb[64:128, p, j1 * 128:(j1 + 1) * 128],
                        rhs=qtb[64:128, p, qsl], start=True, stop=True)
                    pt0 = ptp.tile([128, 2 * F], bf16, tag="pt", name=f"pt0_{jp}")
                    pt1 = ptp.tile([128, 2 * F], bf16, tag="pt", name=f"pt1_{jp}")
                    nc.scalar.activation(pt0, s0, AF.Exp, scale=0.125)
                    nc.vector.tensor_tensor(
                        pt0.rearrange("p (j q) -> p j q", j=2),
                        pt0.rearrange("p (j q) -> p j q", j=2),
                        st["bt0"][:, j0:j0 + 2, :], ALU.mult)
                    nc.scalar.activation(pt1, s1, AF.Exp, scale=0.125)
                    nc.vector.tensor_tensor(
                        pt1.rearrange("p (j q) -> p j q", j=2),
                        pt1.rearrange("p (j q) -> p j q", j=2),
                        st["bt1"][:, j0:j0 + 2, :], ALU.mult)
                    return (p, nbq, jp, pt0, pt1)

                def pv_emit(work):
                    p, nbq, jp, pt0, pt1 = work
                    h0, h1 = 2 * p, 2 * p + 1
                    st = pair_state[(p, nbq)]
                    if jp == 0:
                        # allocate accumulators here: by program order the
                        # previous pair's last access to these PSUM banks
                        # (its evac) has already been emitted
                        st["pv0"] = pvp.tile([D + 1, F], f32, tag="pv",
                                             name=f"pv0_{p}_{nbq}")
                        st["pv1"] = pvp.tile([D + 1, F], f32, tag="pv",
                                             name=f"pv1_{p}_{nbq}")
                    j0, j1 = 2 * jp, 2 * jp + 1
                    nc.tensor.matmul(
                        st["pv0"], lhsT=vb[:, j0, h0, :], rhs=pt0[:, 0:F],
                        start=(jp == 0), stop=False)
                    nc.tensor.matmul(
                        st["pv1"], lhsT=vb[:, j0, h1, :], rhs=pt1[:, 0:F],
                        start=(jp == 0), stop=False)
                    nc.tensor.matmul(
                        st["pv0"], lhsT=vb[:, j1, h0, :], rhs=pt0[:, F:2 * F],
                        start=False, stop=(jp == 3))
                    nc.tensor.matmul(
                        st["pv1"], lhsT=vb[:, j1, h1, :], rhs=pt1[:, F:2 * F],
                        start=False, stop=(jp == 3))
                    if jp == 3:
                        pair_tail(p, nbq)

                def pair_tail(p, nbq):
                    """Reciprocal of softmax sums + fused normalize/evac."""
                    h0, h1 = 2 * p, 2 * p + 1
                    qsl = slice(nbq * F, (nbq + 1) * F)
                    st = pair_state.pop((p, nbq))
                    pv0, pv1 = st["pv0"], st["pv1"]
                    sr0 = rrp.tile([1, F], f32, tag="sr")
                    nc.vector.tensor_copy(sr0, pv0[D:D + 1, :])
                    rr0 = rrp.tile([1, F], f32, tag="rr")
                    nc.vector.reciprocal_approx_fast(rr0, sr0)
                    nc.gpsimd.dma_start(rscr_d[h0, nbq:nbq + 1, :], rr0)
                    rb0 = rbp.tile([128, F], f32, tag="rb")
                    nc.sync.dma_start(
                        rb0, rscr_d[h0, nbq:nbq + 1, :].to_broadcast([128, F]))
                    nc.vector.tensor_tensor(
                        atb[0:64, p, qsl], pv0[0:D, :], rb0[0:64, :], ALU.mult)
                    sr1 = rrp.tile([1, F], f32, tag="sr")
                    nc.vector.tensor_copy(sr1, pv1[D:D + 1, :])
                    rr1 = rrp.tile([1, F], f32, tag="rr")
                    nc.vector.reciprocal_approx_fast(rr1, sr1)
                    nc.gpsimd.dma_start(rscr_d[h1, nbq:nbq + 1, :], rr1)
                    rb1 = rbp.tile([128, F], f32, tag="rb")
                    nc.sync.dma_start(
                        rb1, rscr_d[h1, nbq:nbq + 1, :].to_broadcast([128, F]))
                    vst = vstp.tile([D, F], bf16, tag="vst")
                    nc.vector.tensor_tensor(
                        vst, pv1[0:D, :], rb1[0:64, :], ALU.mult)
                    nc.gpsimd.dma_start(atb[64:128, p, qsl], vst)

                # ---- emission schedule --------------------------------
                bias_load(0, 0)
                # pair-0 essentials, emitted directly
                v_chunk(0, range(0, NT))()
                kq_chunk("k", 0, 0)()
                kq_chunk("k", 0, 1)()
                kq_chunk("q", 0, 0)()

                # filler for nb0 pass: V/K/Q for pairs 1..5, then Q nb1
                for p in range(1, NP):
                    fill_q.append(v_chunk(p, range(0, NT // 2)))
                    fill_q.append(v_chunk(p, range(NT // 2, NT)))
                    fill_q.append(kq_chunk("k", p, 0))
                    fill_q.append(kq_chunk("k", p, 1))
                    fill_q.append(kq_chunk("q", p, 0))
                for p in range(NP):
                    fill_q.append(kq_chunk("q", p, 1))

                # flat slot stream; PV stage lags 2 slots behind scores so
                # ACT never drains at pair boundaries
                slots = [(p, 0, jp) for p in range(NP) for jp in range(4)] + \
                        [(p, 1, jp) for p in range(NP) for jp in range(4)]
                seq = [(p, 0) for p in range(NP)] + [(p, 1) for p in range(NP)]
                pend = []
                for i, (p, nbq, jp) in enumerate(slots):
                    if jp == 0:
                        u = seq.index((p, nbq))
                        if u + 1 < len(seq):
                            bias_load(*seq[u + 1])
                    if nbq == 1 and p == 0 and jp == 2:
                        # emit only after pair-5-nb0's delayed tail (slot 25)
                        # has evacuated atb[:, 5, 0:F]
                        for cot in range(CT):
                            fill_q.append(proj_chunk(cot, 0))
                    fill(2 if i < 28 else 1)
                    work = scores_emit(p, nbq, jp)
                    if len(pend) >= 2:
                        pv_emit(pend.pop(0))
                    pend.append(work)
                while pend:
                    pv_emit(pend.pop(0))
                fill(len(fill_q))
                # tail: projection for q-half 1
                for cot in range(CT):
                    proj_chunk(cot, 1, tail=True)()

    nc.compile()
    return nc


def _get_nc():
    if "nc" not in _cache:
        _cache["nc"] = _build()
    return _cache["nc"]


def prep_in_maps(x, attn_bias, Wq, Wk, Wv, Wp, bp):
    """Host-side sharding + layout prep (transposes/casts only)."""
    wqT = np.ascontiguousarray(Wq.T).astype(BF16)
    wkT = np.ascontiguousarray(Wk.T).astype(BF16)
    wvT = np.ascontiguousarray(Wv.T).astype(BF16)
    wpT = np.ascontiguousarray(Wp.T).astype(BF16)
    bpT = np.ascontiguousarray(bp.astype(np.float32).reshape(CT, 128).T)
    expbT = np.ascontiguousarray(
        np.exp(attn_bias[0].astype(np.float32)).transpose(0, 2, 1)
    ).astype(BF16)
    in_maps = []
    for b in range(B):
        in_maps.append({
            "xT": np.ascontiguousarray(x[b].T).astype(BF16),
            "wqT": wqT, "wkT": wkT, "wvT": wvT, "wpT": wpT,
            "bpT": bpT, "expbT": expbT,
        })
    return in_maps


def run(in_maps, trace=False, **kw):
    from concourse.bass_utils import run_bass_kernel_spmd

    nc = _get_nc()
    return run_bass_kernel_spmd(
        nc, in_maps, core_ids=list(range(B)), trace=trace, **kw
    )


def kernel(x, attn_bias, Wq, Wk, Wv, Wp, bp):
    res = run(prep_in_maps(x, attn_bias, Wq, Wk, Wv, Wp, bp))
    out = np.stack(
        [res.results[b]["outT"].T for b in range(B)]
    ).astype(np.float32)
    return out


# revision 11
# speedup vs baseline: 1.2009x; 1.0214x over previous
"""Multi-head attention (B=8, N=1024, C=768, H=12, D=64) on 8 TRN2 NeuronCores.

Strategy: data-parallel over batch (B == n_cores == 8), no collectives.
v2 design, fully transposed layout (channels on SBUF partitions):

  - Scores: heads processed in pairs (h0=2p on partitions 0:64, h1=2p+1 on
    64:128).  The two heads' score matmuls (contraction K=D=64) are emitted
    back-to-back with row tile_positions (0,0)/(64,0), so the PE runs them
    concurrently in separate array halves -> ~2x score throughput.
  - Score PSUM tiles are [128, 1024] (two banks: key-tiles 2jp, 2jp+1), so a
    single ACT exp instruction covers 1024 elements, amortizing overhead.
  - Bias handled as P = exp(0.125*S^T) * exp(bias)^T: exp(bias) precomputed
    host-side (bf16), applied on DVE in 2x-rate 16-bit mode (cheaper than the
    f32 PSUM add at 1x).
  - PV keeps the ones-row trick (lhsT = [V_h | 1], M=65) for softmax sums.
  - Normalization per (pair, nb-half): DVE reciprocal of the PSUM sum row,
    DMA-broadcast across partitions, fused into the PV evacuation multiply.
  - Loop order: nb (query half) outer, head-pair inner.  Bias DMA is split
    per (head, nb) so each byte is loaded once.  Output projection for
    q-half 0 runs as PE filler during q-half 1's attention.
  - QKV projections + output projection are emitted as "filler" chunks
    inside the ACT-bound attention loop to keep the PE dense (HAM warm).
"""

import os
import sys
import numpy as np

for _p in ("/opt/trn_rl_repo", "/root/.axon_site/_ro/trn_rl_repo"):
    if os.path.isdir(_p) and _p not in sys.path:
        sys.path.append(_p)

import ml_dtypes

BF16 = ml_dtypes.bfloat16

B, N, C = 8, 1024, 768
H, D = 12, 64
CT = C // 128        # 6 channel tiles
NT = N // 128        # 8 key tiles
F = 512
NP = H // 2          # 6 head pairs

_cache = {}


def _build():
    import concourse.bass as bass
    import concourse.tile as tile
    from concourse import bacc, mybir

    f32 = mybir.dt.float32
    bf16 = mybir.dt.bfloat16
    AF = mybir.ActivationFunctionType
    ALU = mybir.AluOpType

    nc = bacc.Bacc("TRN2", target_bir_lowering=False)

    xT_d = nc.dram_tensor("xT", [C, N], bf16, kind="ExternalInput")
    wqT_d = nc.dram_tensor("wqT", [C, C], bf16, kind="ExternalInput")
    wkT_d = nc.dram_tensor("wkT", [C, C], bf16, kind="ExternalInput")
    wvT_d = nc.dram_tensor("wvT", [C, C], bf16, kind="ExternalInput")
    wpT_d = nc.dram_tensor("wpT", [C, C], bf16, kind="ExternalInput")
    bpT_d = nc.dram_tensor("bpT", [128, CT], f32, kind="ExternalInput")
    expbT_d = nc.dram_tensor("expbT", [H, N, N], bf16, kind="ExternalInput")
    outT_d = nc.dram_tensor("outT", [C, N], f32, kind="ExternalOutput")
    rscr_d = nc.dram_tensor("rscr", [H, 2, F], f32)  # reciprocal bounce

    with tile.TileContext(nc) as tc:
        with tc.tile_pool(name="persist", bufs=1) as pers:
            xTb = pers.tile([128, CT, N], bf16, tag="xT")
            wqb = pers.tile([128, CT, C], bf16, tag="wq")
            wkb = pers.tile([128, CT, C], bf16, tag="wk")
            wvb = pers.tile([128, CT, C], bf16, tag="wv")
            wpb = pers.tile([128, CT, C], bf16, tag="wp")
            bpb = pers.tile([128, CT], f32, tag="bp")
            qtb = pers.tile([128, CT, N], bf16, tag="qt")
            ktb = pers.tile([128, CT, N], bf16, tag="kt")
            vb = pers.tile([128, NT, H, D + 1], bf16, tag="v")
            atb = pers.tile([128, CT, N], bf16, tag="at")
            dum = pers.tile([1, 8], f32, tag="dum")

            # ---- phase 0: input DMAs (pair-0 essentials first) -----------
            for i, q0 in enumerate(range(0, N, 256)):
                eng = nc.sync if i % 2 == 0 else nc.scalar
                eng.dma_start(
                    xTb[:, :, q0:q0 + 256],
                    xT_d[:, q0:q0 + 256].rearrange("(ci p) n -> p ci n", p=128))
            nc.scalar.dma_start(
                wvb[:, :, 0:128],
                wvT_d[:, 0:128].rearrange("(ci p) o -> p ci o", p=128))
            nc.sync.dma_start(
                wkb[:, :, 0:128],
                wkT_d[:, 0:128].rearrange("(ci p) o -> p ci o", p=128))
            nc.scalar.dma_start(
                wqb[:, :, 0:128],
                wqT_d[:, 0:128].rearrange("(ci p) o -> p ci o", p=128))
            nc.scalar.dma_start(bpb, bpT_d[:])
            nc.sync.dma_start(
                wvb[:, :, 128:C],
                wvT_d[:, 128:C].rearrange("(ci p) o -> p ci o", p=128))
            nc.scalar.dma_start(
                wkb[:, :, 128:C],
                wkT_d[:, 128:C].rearrange("(ci p) o -> p ci o", p=128))
            nc.sync.dma_start(
                wqb[:, :, 128:C],
                wqT_d[:, 128:C].rearrange("(ci p) o -> p ci o", p=128))
            nc.scalar.dma_start(
                wpb[:, :, 0:384],
                wpT_d[:, 0:384].rearrange("(ci p) o -> p ci o", p=128))
            nc.sync.dma_start(
                wpb[:, :, 384:C],
                wpT_d[:, 384:C].rearrange("(ci p) o -> p ci o", p=128))

            nc.vector.memset(vb[:, :, :, D:D + 1], 1.0)
            nc.vector.memset(dum, 1.0)
            nc.scalar.activation(dum, dum, AF.Exp)  # preload exp table set

            with tc.tile_pool(name="sps", bufs=2, space="PSUM") as sp, \
                 tc.tile_pool(name="pvps", bufs=2, space="PSUM") as pvp, \
                 tc.tile_pool(name="ups", bufs=2, space="PSUM") as ups, \
                 tc.tile_pool(name="ptp", bufs=6) as ptp, \
                 tc.tile_pool(name="btp", bufs=4) as btp, \
                 tc.tile_pool(name="rbp", bufs=4) as rbp, \
                 tc.tile_pool(name="rrp", bufs=4) as rrp, \
                 tc.tile_pool(name="vstp", bufs=2) as vstp, \
                 tc.tile_pool(name="otp", bufs=3) as otp:

                cp_state = [0, False]  # [rotation, attention_started]

                def cp(dst, src):
                    """PSUM->SBUF copy.  DVE-only once attention starts
                    (an ACT-queued copy behind EXPs stalls ups recycling)."""
                    if cp_state[1]:
                        nc.vector.tensor_copy(dst, src)
                        return
                    cp_state[0] = (cp_state[0] + 1) % 3
                    if cp_state[0]:
                        nc.vector.tensor_copy(dst, src)
                    else:
                        nc.scalar.copy(dst, src)

                def v_chunk(p, nts):
                    """V projection for heads 2p, 2p+1 over key tiles nts."""
                    f0 = p * 128

                    def go():
                        for nt in nts:
                            ps = ups.tile([128, F], f32, tag="u")
                            for ci in range(CT):
                                nc.tensor.matmul(
                                    ps[:, 0:128],
                                    lhsT=xTb[:, ci, nt * 128:(nt + 1) * 128],
                                    rhs=wvb[:, ci, f0:f0 + 128],
                                    start=(ci == 0),
                                    stop=(ci == CT - 1),
                                )
                            cp(vb[:, nt, 2 * p:2 * p + 2, 0:D],
                               ps[:, 0:128].rearrange("p (h d) -> p h d", d=D))
                    return go

                def kq_chunk(which, cot, nbq):
                    wb, dst = (wkb, ktb) if which == "k" else (wqb, qtb)

                    def go():
                        ps = ups.tile([128, F], f32, tag="u")
                        for ci in range(CT):
                            nc.tensor.matmul(
                                ps,
                                lhsT=wb[:, ci, cot * 128:(cot + 1) * 128],
                                rhs=xTb[:, ci, nbq * F:(nbq + 1) * F],
                                start=(ci == 0),
                                stop=(ci == CT - 1),
                            )
                        cp(dst[:, cot, nbq * F:(nbq + 1) * F], ps)
                    return go

                def proj_chunk(cot, nbq, tail=False):
                    def go():
                        ps = ups.tile([128, F], f32, tag="u")
                        for ci in range(CT):
                            nc.tensor.matmul(
                                ps,
                                lhsT=wpb[:, ci, cot * 128:(cot + 1) * 128],
                                rhs=atb[:, ci, nbq * F:(nbq + 1) * F],
                                start=(ci == 0),
                                stop=(ci == CT - 1),
                            )
                        ot = otp.tile([128, F], f32, tag="ot")
                        if tail:
                            nc.scalar.activation(
                                ot, ps, AF.Identity, bias=bpb[:, cot:cot + 1])
                        else:
                            nc.vector.tensor_scalar_add(
                                ot, ps, bpb[:, cot:cot + 1])
                        eng = nc.sync if cot % 2 else nc.scalar
                        eng.dma_start(
                            outT_d[cot * 128:(cot + 1) * 128,
                                   nbq * F:(nbq + 1) * F],
                            ot,
                        )
                    return go

                fill_q = []

                def fill(n):
                    for _ in range(n):
                        if fill_q:
                            fill_q.pop(0)()

                bt_map = {}

                def bias_load(p, nbq):
                    for h in (2 * p, 2 * p + 1):
                        bt = btp.tile([128, NT, F], bf16, tag="bt")
                        src = expbT_d[h].rearrange("(j p) q -> p j q", p=128)
                        nc.sync.dma_start(
                            bt[:, 0:NT // 2, :],
                            src[:, 0:NT // 2, nbq * F:(nbq + 1) * F])
                        nc.sync.dma_start(
                            bt[:, NT // 2:NT, :],
                            src[:, NT // 2:NT, nbq * F:(nbq + 1) * F])
                        bt_map[(h, nbq)] = bt

                pair_state = {}

                def scores_emit(p, nbq, jp):
                    """Score matmuls + exp + bias-mult for one jp slot."""
                    h0, h1 = 2 * p, 2 * p + 1
                    qsl = slice(nbq * F, (nbq + 1) * F)
                    if jp == 0:
                        pair_state[(p, nbq)] = {
                            "bt0": bt_map.pop((h0, nbq)),
                            "bt1": bt_map.pop((h1, nbq)),
                        }
                    st = pair_state[(p, nbq)]
                    j0, j1 = 2 * jp, 2 * jp + 1
                    s0 = sp.tile([128, 2 * F], f32, tag="s", name=f"s0_{jp}")
                    s1 = sp.tile([128, 2 * F], f32, tag="s", name=f"s1_{jp}")
                    # row-tile pairs: (0,0) and (64,0) run concurrently
                    nc.tensor.matmul(
                        s0[:, 0:F],
                        lhsT=ktb[0:64, p, j0 * 128:(j0 + 1) * 128],
                        rhs=qtb[0:64, p, qsl], start=True, stop=True)
                    nc.tensor.matmul(
                        s1[:, 0:F],
                        lhsT=ktb[64:128, p, j0 * 128:(j0 + 1) * 128],
                        rhs=qtb[64:128, p, qsl], start=True, stop=True)
                    nc.tensor.matmul(
                        s0[:, F:2 * F],
                        lhsT=ktb[0:64, p, j1 * 128:(j1 + 1) * 128],
                        rhs=qtb[0:64, p, qsl], start=True, stop=True)
                    nc.tensor.matmul(
                        s1[:, F:2 * F],
                        lhsT=ktb[64:128, p, j1 * 128:(j1 + 1) * 128],
                        rhs=qtb[64:128, p, qsl], start=True, stop=True)
                    pt0 = ptp.tile([128, 2 * F], bf16, tag="pt", name=f"pt0_{jp}")
                    pt1 = ptp.tile([128, 2 * F], bf16, tag="pt", name=f"pt1_{jp}")
                    nc.scalar.activation(pt0, s0, AF.Exp, scale=0.125)
                    nc.vector.tensor_tensor(
                        pt0.rearrange("p (j q) -> p j q", j=2),
                        pt0.rearrange("p (j q) -> p j q", j=2),
                        st["bt0"][:, j0:j0 + 2, :], ALU.mult)
                    nc.scalar.activation(pt1, s1, AF.Exp, scale=0.125)
                    nc.vector.tensor_tensor(
                        pt1.rearrange("p (j q) -> p j q", j=2),
                        pt1.rearrange("p (j q) -> p j q", j=2),
                        st["bt1"][:, j0:j0 + 2, :], ALU.mult)
                    return (p, nbq, jp, pt0, pt1)

                def pv_emit(work):
                    p, nbq, jp, pt0, pt1 = work
                    h0, h1 = 2 * p, 2 * p + 1
                    st = pair_state[(p, nbq)]
                    if jp == 0:
                        # allocate accumulators here: by program order the
                        # previous pair's last access to these PSUM banks
                        # (its evac) has already been emitted
                        st["pv0"] = pvp.tile([D + 1, F], f32, tag="pv",
                                             name=f"pv0_{p}_{nbq}")
                        st["pv1"] = pvp.tile([D + 1, F], f32, tag="pv",
                                             name=f"pv1_{p}_{nbq}")
                    j0, j1 = 2 * jp, 2 * jp + 1
                    nc.tensor.matmul(
                        st["pv0"], lhsT=vb[:, j0, h0, :], rhs=pt0[:, 0:F],
                        start=(jp == 0), stop=False)
                    nc.tensor.matmul(
                        st["pv1"], lhsT=vb[:, j0, h1, :], rhs=pt1[:, 0:F],
                        start=(jp == 0), stop=False)
                    nc.tensor.matmul(
                        st["pv0"], lhsT=vb[:, j1, h0, :], rhs=pt0[:, F:2 * F],
                        start=False, stop=(jp == 3))
                    nc.tensor.matmul(
                        st["pv1"], lhsT=vb[:, j1, h1, :], rhs=pt1[:, F:2 * F],
                        start=False, stop=(jp == 3))
                    if jp == 3:
                        pair_tail(p, nbq)

                def pair_tail(p, nbq):
                    """Reciprocal of softmax sums + fused normalize/evac."""
                    h0, h1 = 2 * p, 2 * p + 1
                    qsl = slice(nbq * F, (nbq + 1) * F)
                    st = pair_state.pop((p, nbq))
                    pv0, pv1 = st["pv0"], st["pv1"]
                    sr0 = rrp.tile([1, F], f32, tag="sr")
                    nc.vector.tensor_copy(sr0, pv0[D:D + 1, :])
                    rr0 = rrp.tile([1, F], f32, tag="rr")
                    nc.vector.reciprocal_approx_fast(rr0, sr0)
                    nc.gpsimd.dma_start(rscr_d[h0, nbq:nbq + 1, :], rr0)
                    rb0 = rbp.tile([128, F], f32, tag="rb")
                    nc.sync.dma_start(
                        rb0, rscr_d[h0, nbq:nbq + 1, :].to_broadcast([128, F]))
                    nc.vector.tensor_tensor(
                        atb[0:64, p, qsl], pv0[0:D, :], rb0[0:64, :], ALU.mult)
                    sr1 = rrp.tile([1, F], f32, tag="sr")
                    nc.vector.tensor_copy(sr1, pv1[D:D + 1, :])
                    rr1 = rrp.tile([1, F], f32, tag="rr")
                    nc.vector.reciprocal_approx_fast(rr1, sr1)
                    nc.gpsimd.dma_start(rscr_d[h1, nbq:nbq + 1, :], rr1)
                    rb1 = rbp.tile([128, F], f32, tag="rb")
                    nc.sync.dma_start(
                        rb1, rscr_d[h1, nbq:nbq + 1, :].to_broadcast([128, F]))
                    vst = vstp.tile([D, F], bf16, tag="vst")
                    nc.vector.tensor_tensor(
                        vst, pv1[0:D, :], rb1[0:64, :], ALU.mult)
                    nc.gpsimd.dma_start(atb[64:128, p, qsl], vst)

                # ---- emission schedule --------------------------------
                bias_load(0, 0)
                # pair-0 essentials, emitted directly
                v_chunk(0, range(0, NT))()
                kq_chunk("k", 0, 0)()
                kq_chunk("k", 0, 1)()
                kq_chunk("q", 0, 0)()

                # filler for nb0 pass: V/K/Q for pairs 1..5, then Q nb1
                for p in range(1, NP):
                    fill_q.append(v_chunk(p, range(0, NT // 2)))
                    fill_q.append(v_chunk(p, range(NT // 2, NT)))
                    fill_q.append(kq_chunk("k", p, 0))
                    fill_q.append(kq_chunk("k", p, 1))
                    fill_q.append(kq_chunk("q", p, 0))
                for p in range(NP):
                    fill_q.append(kq_chunk("q", p, 1))

                # flat slot stream; PV stage lags 2 slots behind scores so
                # ACT never drains at pair boundaries
                slots = [(p, 0, jp) for p in range(NP) for jp in range(4)] + \
                        [(p, 1, jp) for p in range(NP) for jp in range(4)]
                seq = [(p, 0) for p in range(NP)] + [(p, 1) for p in range(NP)]
                pend = []
                for i, (p, nbq, jp) in enumerate(slots):
                    if jp == 0:
                        u = seq.index((p, nbq))
                        if u + 1 < len(seq):
                            bias_load(*seq[u + 1])
                    if nbq == 1 and jp == 2 and p in (0, 1, 2):
                        # only after pair-5-nb0's delayed tail (slot 25) has
                        # evacuated atb[:, 5, 0:F]
                        for cot in (2 * p, 2 * p + 1):
                            fill_q.append(proj_chunk(cot, 0))
                    work = scores_emit(p, nbq, jp)
                    if len(pend) >= 2:
                        pv_emit(pend.pop(0))
                    pend.append(work)
                    if i == 0:
                        cp_state[1] = True
                    fill(2 if i < 28 else 1)
                while pend:
                    pv_emit(pend.pop(0))
                fill(len(fill_q))
                # tail: projection for q-half 1
                for cot in range(CT):
                    proj_chunk(cot, 1, tail=True)()

    nc.compile()
    return nc


def _get_nc():
    if "nc" not in _cache:
        _cache["nc"] = _build()
    return _cache["nc"]


def prep_in_maps(x, attn_bias, Wq, Wk, Wv, Wp, bp):
    """Host-side sharding + layout prep (transposes/casts only)."""
    wqT = np.ascontiguousarray(Wq.T).astype(BF16)
    wkT = np.ascontiguousarray(Wk.T).astype(BF16)
    wvT = np.ascontiguousarray(Wv.T).astype(BF16)
    wpT = np.ascontiguousarray(Wp.T).astype(BF16)
    bpT = np.ascontiguousarray(bp.astype(np.float32).reshape(CT, 128).T)
    expbT = np.ascontiguousarray(
        np.exp(attn_bias[0].astype(np.float32)).transpose(0, 2, 1)
    ).astype(BF16)
    in_maps = []
    for b in range(B):
        in_maps.append({
            "xT": np.ascontiguousarray(x[b].T).astype(BF16),
            "wqT": wqT, "wkT": wkT, "wvT": wvT, "wpT": wpT,
            "bpT": bpT, "expbT": expbT,
        })
    return in_maps


def run(in_maps, trace=False, **kw):
    from concourse.bass_utils import run_bass_kernel_spmd

    nc = _get_nc()
    return run_bass_kernel_spmd(
        nc, in_maps, core_ids=list(range(B)), trace=trace, **kw
    )


def kernel(x, attn_bias, Wq, Wk, Wv, Wp, bp):
    res = run(prep_in_maps(x, attn_bias, Wq, Wk, Wv, Wp, bp))
    out = np.stack(
        [res.results[b]["outT"].T for b in range(B)]
    ).astype(np.float32)
    return out


# revision 12
# speedup vs baseline: 1.2781x; 1.0643x over previous
"""Multi-head attention (B=8, N=1024, C=768, H=12, D=64) on 8 TRN2 NeuronCores.

Strategy: data-parallel over batch (B == n_cores == 8), no collectives.
v2 design, fully transposed layout (channels on SBUF partitions):

  - Scores: heads processed in pairs (h0=2p on partitions 0:64, h1=2p+1 on
    64:128).  The two heads' score matmuls (contraction K=D=64) are emitted
    back-to-back with row tile_positions (0,0)/(64,0), so the PE runs them
    concurrently in separate array halves -> ~2x score throughput.
  - Score PSUM tiles are [128, 1024] (two banks: key-tiles 2jp, 2jp+1), so a
    single ACT exp instruction covers 1024 elements, amortizing overhead.
  - Bias handled as P = exp(0.125*S^T) * exp(bias)^T: exp(bias) precomputed
    host-side (bf16), applied on DVE in 2x-rate 16-bit mode (cheaper than the
    f32 PSUM add at 1x).
  - PV keeps the ones-row trick (lhsT = [V_h | 1], M=65) for softmax sums.
  - Normalization per (pair, nb-half): DVE reciprocal of the PSUM sum row,
    DMA-broadcast across partitions, fused into the PV evacuation multiply.
  - Loop order: nb (query half) outer, head-pair inner.  Bias DMA is split
    per (head, nb) so each byte is loaded once.  Output projection for
    q-half 0 runs as PE filler during q-half 1's attention.
  - QKV projections + output projection are emitted as "filler" chunks
    inside the ACT-bound attention loop to keep the PE dense (HAM warm).
"""

import os
import sys
import numpy as np

for _p in ("/opt/trn_rl_repo", "/root/.axon_site/_ro/trn_rl_repo"):
    if os.path.isdir(_p) and _p not in sys.path:
        sys.path.append(_p)

import ml_dtypes

BF16 = ml_dtypes.bfloat16

B, N, C = 8, 1024, 768
H, D = 12, 64
CT = C // 128        # 6 channel tiles
NT = N // 128        # 8 key tiles
F = 512
NP = H // 2          # 6 head pairs

_cache = {}


def _build():
    import concourse.bass as bass
    import concourse.tile as tile
    from concourse import bacc, mybir

    f32 = mybir.dt.float32
    bf16 = mybir.dt.bfloat16
    AF = mybir.ActivationFunctionType
    ALU = mybir.AluOpType

    nc = bacc.Bacc("TRN2", target_bir_lowering=False)

    xT_d = nc.dram_tensor("xT", [C, N], bf16, kind="ExternalInput")
    wqT_d = nc.dram_tensor("wqT", [C, C], bf16, kind="ExternalInput")
    wkT_d = nc.dram_tensor("wkT", [C, C], bf16, kind="ExternalInput")
    wvT_d = nc.dram_tensor("wvT", [C, C], bf16, kind="ExternalInput")
    wpT_d = nc.dram_tensor("wpT", [C, C], bf16, kind="ExternalInput")
    bpT_d = nc.dram_tensor("bpT", [128, CT], f32, kind="ExternalInput")
    expbT_d = nc.dram_tensor("expbT", [H, N, N], bf16, kind="ExternalInput")
    outT_d = nc.dram_tensor("outT", [C, N], f32, kind="ExternalOutput")
    rscr_d = nc.dram_tensor("rscr", [H, 2, F], f32)  # reciprocal bounce

    with tile.TileContext(nc) as tc:
        with tc.tile_pool(name="persist", bufs=1) as pers:
            xTb = pers.tile([128, CT, N], bf16, tag="xT")
            wqb = pers.tile([128, CT, C], bf16, tag="wq")
            wkb = pers.tile([128, CT, C], bf16, tag="wk")
            wvb = pers.tile([128, CT, C], bf16, tag="wv")
            wpb = pers.tile([128, CT, C], bf16, tag="wp")
            bpb = pers.tile([128, CT], f32, tag="bp")
            qtb = pers.tile([128, CT, N], bf16, tag="qt")
            ktb = pers.tile([128, CT, N], bf16, tag="kt")
            vb = pers.tile([128, NT, H, D + 1], bf16, tag="v")
            atb = pers.tile([128, CT, N], bf16, tag="at")
            dum = pers.tile([1, 8], f32, tag="dum")

            # ---- phase 0: input DMAs (pair-0 essentials first) -----------
            for i, q0 in enumerate(range(0, N, 256)):
                eng = nc.sync if i % 2 == 0 else nc.scalar
                eng.dma_start(
                    xTb[:, :, q0:q0 + 256],
                    xT_d[:, q0:q0 + 256].rearrange("(ci p) n -> p ci n", p=128))
            nc.scalar.dma_start(
                wvb[:, :, 0:128],
                wvT_d[:, 0:128].rearrange("(ci p) o -> p ci o", p=128))
            nc.sync.dma_start(
                wkb[:, :, 0:128],
                wkT_d[:, 0:128].rearrange("(ci p) o -> p ci o", p=128))
            nc.scalar.dma_start(
                wqb[:, :, 0:128],
                wqT_d[:, 0:128].rearrange("(ci p) o -> p ci o", p=128))
            nc.scalar.dma_start(bpb, bpT_d[:])
            nc.sync.dma_start(
                wvb[:, :, 128:C],
                wvT_d[:, 128:C].rearrange("(ci p) o -> p ci o", p=128))
            nc.scalar.dma_start(
                wkb[:, :, 128:C],
                wkT_d[:, 128:C].rearrange("(ci p) o -> p ci o", p=128))
            nc.sync.dma_start(
                wqb[:, :, 128:C],
                wqT_d[:, 128:C].rearrange("(ci p) o -> p ci o", p=128))
            nc.scalar.dma_start(
                wpb[:, :, 0:384],
                wpT_d[:, 0:384].rearrange("(ci p) o -> p ci o", p=128))
            nc.sync.dma_start(
                wpb[:, :, 384:C],
                wpT_d[:, 384:C].rearrange("(ci p) o -> p ci o", p=128))

            nc.vector.memset(vb[:, :, :, D:D + 1], 1.0)
            nc.vector.memset(dum, 1.0)
            nc.scalar.activation(dum, dum, AF.Exp)  # preload exp table set

            with tc.tile_pool(name="sps", bufs=2, space="PSUM") as sp, \
                 tc.tile_pool(name="pvps", bufs=2, space="PSUM") as pvp, \
                 tc.tile_pool(name="ups", bufs=2, space="PSUM") as ups, \
                 tc.tile_pool(name="ptp", bufs=6) as ptp, \
                 tc.tile_pool(name="btp", bufs=4) as btp, \
                 tc.tile_pool(name="rbp", bufs=4) as rbp, \
                 tc.tile_pool(name="rrp", bufs=4) as rrp, \
                 tc.tile_pool(name="vstp", bufs=2) as vstp, \
                 tc.tile_pool(name="otp", bufs=3) as otp:

                cp_state = [0, False]  # [rotation, attention_started]

                def cp(dst, src):
                    """PSUM->SBUF copy.  DVE-only once attention starts
                    (an ACT-queued copy behind EXPs stalls ups recycling)."""
                    if cp_state[1]:
                        nc.vector.tensor_copy(dst, src)
                        return
                    cp_state[0] = (cp_state[0] + 1) % 3
                    if cp_state[0]:
                        nc.vector.tensor_copy(dst, src)
                    else:
                        nc.scalar.copy(dst, src)

                def v_chunk(p, nts):
                    """V projection for heads 2p, 2p+1 over key tiles nts."""
                    f0 = p * 128

                    def go():
                        for nt in nts:
                            ps = ups.tile([128, F], f32, tag="u")
                            for ci in range(CT):
                                nc.tensor.matmul(
                                    ps[:, 0:128],
                                    lhsT=xTb[:, ci, nt * 128:(nt + 1) * 128],
                                    rhs=wvb[:, ci, f0:f0 + 128],
                                    start=(ci == 0),
                                    stop=(ci == CT - 1),
                                )
                            cp(vb[:, nt, 2 * p:2 * p + 2, 0:D],
                               ps[:, 0:128].rearrange("p (h d) -> p h d", d=D))
                    return go

                def kq_chunk(which, cot, nbq):
                    wb, dst = (wkb, ktb) if which == "k" else (wqb, qtb)

                    def go():
                        ps = ups.tile([128, F], f32, tag="u")
                        for ci in range(CT):
                            nc.tensor.matmul(
                                ps,
                                lhsT=wb[:, ci, cot * 128:(cot + 1) * 128],
                                rhs=xTb[:, ci, nbq * F:(nbq + 1) * F],
                                start=(ci == 0),
                                stop=(ci == CT - 1),
                            )
                        cp(dst[:, cot, nbq * F:(nbq + 1) * F], ps)
                    return go

                def proj_chunk(cot, nbq, tail=False):
                    def go():
                        ps = ups.tile([128, F], f32, tag="u")
                        for ci in range(CT):
                            nc.tensor.matmul(
                                ps,
                                lhsT=wpb[:, ci, cot * 128:(cot + 1) * 128],
                                rhs=atb[:, ci, nbq * F:(nbq + 1) * F],
                                start=(ci == 0),
                                stop=(ci == CT - 1),
                            )
                        ot = otp.tile([128, F], f32, tag="ot")
                        if tail:
                            nc.scalar.activation(
                                ot, ps, AF.Identity, bias=bpb[:, cot:cot + 1])
                        else:
                            nc.vector.tensor_scalar_add(
                                ot, ps, bpb[:, cot:cot + 1])
                        eng = nc.sync if cot % 2 else nc.scalar
                        eng.dma_start(
                            outT_d[cot * 128:(cot + 1) * 128,
                                   nbq * F:(nbq + 1) * F],
                            ot,
                        )
                    return go

                fill_q = []

                def fill(n):
                    for _ in range(n):
                        if fill_q:
                            fill_q.pop(0)()

                bt_map = {}

                def bias_load(p, nbq):
                    for h in (2 * p, 2 * p + 1):
                        bt = btp.tile([128, NT, F], bf16, tag="bt")
                        src = expbT_d[h].rearrange("(j p) q -> p j q", p=128)
                        nc.sync.dma_start(
                            bt[:, 0:NT // 2, :],
                            src[:, 0:NT // 2, nbq * F:(nbq + 1) * F])
                        nc.sync.dma_start(
                            bt[:, NT // 2:NT, :],
                            src[:, NT // 2:NT, nbq * F:(nbq + 1) * F])
                        bt_map[(h, nbq)] = bt

                pair_state = {}

                def scores_emit(p, nbq, jp):
                    """Score matmuls + exp + bias-mult for one jp slot."""
                    h0, h1 = 2 * p, 2 * p + 1
                    qsl = slice(nbq * F, (nbq + 1) * F)
                    if jp == 0:
                        pair_state[(p, nbq)] = {
                            "bt0": bt_map.pop((h0, nbq)),
                            "bt1": bt_map.pop((h1, nbq)),
                        }
                    st = pair_state[(p, nbq)]
                    j0, j1 = 2 * jp, 2 * jp + 1
                    s0 = sp.tile([128, 2 * F], f32, tag="s", name=f"s0_{jp}")
                    s1 = sp.tile([128, 2 * F], f32, tag="s", name=f"s1_{jp}")
                    # row-tile pairs: (0,0) and (64,0) run concurrently
                    nc.tensor.matmul(
                        s0[:, 0:F],
                        lhsT=ktb[0:64, p, j0 * 128:(j0 + 1) * 128],
                        rhs=qtb[0:64, p, qsl], start=True, stop=True)
                    nc.tensor.matmul(
                        s1[:, 0:F],
                        lhsT=ktb[64:128, p, j0 * 128:(j0 + 1) * 128],
                        rhs=qtb[64:128, p, qsl], start=True, stop=True)
                    nc.tensor.matmul(
                        s0[:, F:2 * F],
                        lhsT=ktb[0:64, p, j1 * 128:(j1 + 1) * 128],
                        rhs=qtb[0:64, p, qsl], start=True, stop=True)
                    nc.tensor.matmul(
                        s1[:, F:2 * F],
                        lhsT=ktb[64:128, p, j1 * 128:(j1 + 1) * 128],
                        rhs=qtb[64:128, p, qsl], start=True, stop=True)
                    pt0 = ptp.tile([128, 2 * F], bf16, tag="pt", name=f"pt0_{jp}")
                    pt1 = ptp.tile([128, 2 * F], bf16, tag="pt", name=f"pt1_{jp}")
                    nc.scalar.activation(pt0, s0, AF.Exp, scale=0.125)
                    nc.vector.tensor_tensor(
                        pt0.rearrange("p (j q) -> p j q", j=2),
                        pt0.rearrange("p (j q) -> p j q", j=2),
                        st["bt0"][:, j0:j0 + 2, :], ALU.mult)
                    nc.scalar.activation(pt1, s1, AF.Exp, scale=0.125)
                    nc.vector.tensor_tensor(
                        pt1.rearrange("p (j q) -> p j q", j=2),
                        pt1.rearrange("p (j q) -> p j q", j=2),
                        st["bt1"][:, j0:j0 + 2, :], ALU.mult)
                    return (p, nbq, jp, pt0, pt1)

                def pv_emit(work):
                    p, nbq, jp, pt0, pt1 = work
                    h0, h1 = 2 * p, 2 * p + 1
                    st = pair_state[(p, nbq)]
                    if jp == 0:
                        # allocate accumulators here: by program order the
                        # previous pair's last access to these PSUM banks
                        # (its evac) has already been emitted
                        st["pv0"] = pvp.tile([D + 1, F], f32, tag="pv",
                                             name=f"pv0_{p}_{nbq}")
                        st["pv1"] = pvp.tile([D + 1, F], f32, tag="pv",
                                             name=f"pv1_{p}_{nbq}")
                    j0, j1 = 2 * jp, 2 * jp + 1
                    nc.tensor.matmul(
                        st["pv0"], lhsT=vb[:, j0, h0, :], rhs=pt0[:, 0:F],
                        start=(jp == 0), stop=False)
                    nc.tensor.matmul(
                        st["pv1"], lhsT=vb[:, j0, h1, :], rhs=pt1[:, 0:F],
                        start=(jp == 0), stop=False)
                    nc.tensor.matmul(
                        st["pv0"], lhsT=vb[:, j1, h0, :], rhs=pt0[:, F:2 * F],
                        start=False, stop=(jp == 3))
                    nc.tensor.matmul(
                        st["pv1"], lhsT=vb[:, j1, h1, :], rhs=pt1[:, F:2 * F],
                        start=False, stop=(jp == 3))
                    if jp == 3:
                        pair_tail(p, nbq, last=(p == NP - 1 and nbq == 1))

                def pair_tail(p, nbq, last=False):
                    """Evacuate unnormalized (frees PSUM fast), then
                    reciprocal + in-place normalization off the critical
                    path (GpSimd; DVE for the final unit's tail)."""
                    h0, h1 = 2 * p, 2 * p + 1
                    qsl = slice(nbq * F, (nbq + 1) * F)
                    st = pair_state.pop((p, nbq))
                    pv0, pv1 = st["pv0"], st["pv1"]
                    sr0 = rrp.tile([1, F], f32, tag="sr")
                    nc.vector.tensor_copy(sr0, pv0[D:D + 1, :])
                    nc.vector.tensor_copy(atb[0:64, p, qsl], pv0[0:D, :])
                    sr1 = rrp.tile([1, F], f32, tag="sr")
                    nc.vector.tensor_copy(sr1, pv1[D:D + 1, :])
                    vst = vstp.tile([D, F], bf16, tag="vst")
                    nc.vector.tensor_copy(vst, pv1[0:D, :])
                    nc.gpsimd.dma_start(atb[64:128, p, qsl], vst)
                    rr0 = rrp.tile([1, F], f32, tag="rr")
                    nc.vector.reciprocal_approx_fast(rr0, sr0)
                    nc.gpsimd.dma_start(rscr_d[h0, nbq:nbq + 1, :], rr0)
                    rb0 = rbp.tile([128, F], f32, tag="rb")
                    nc.sync.dma_start(
                        rb0, rscr_d[h0, nbq:nbq + 1, :].to_broadcast([128, F]))
                    rr1 = rrp.tile([1, F], f32, tag="rr")
                    nc.vector.reciprocal_approx_fast(rr1, sr1)
                    nc.gpsimd.dma_start(rscr_d[h1, nbq:nbq + 1, :], rr1)
                    rb1 = rbp.tile([128, F], f32, tag="rb")
                    nc.sync.dma_start(
                        rb1, rscr_d[h1, nbq:nbq + 1, :].to_broadcast([128, F]))
                    eng = nc.vector if last else nc.gpsimd
                    eng.tensor_tensor(
                        atb[0:64, p, qsl], atb[0:64, p, qsl],
                        rb0[0:64, :], ALU.mult)
                    eng.tensor_tensor(
                        atb[64:128, p, qsl], atb[64:128, p, qsl],
                        rb1[64:128, :], ALU.mult)

                # ---- emission schedule --------------------------------
                bias_load(0, 0)
                # pair-0 essentials, emitted directly
                v_chunk(0, range(0, NT))()
                kq_chunk("k", 0, 0)()
                kq_chunk("k", 0, 1)()
                kq_chunk("q", 0, 0)()

                # filler for nb0 pass: V/K/Q for pairs 1..5, then Q nb1
                for p in range(1, NP):
                    fill_q.append(v_chunk(p, range(0, NT // 2)))
                    fill_q.append(v_chunk(p, range(NT // 2, NT)))
                    fill_q.append(kq_chunk("k", p, 0))
                    fill_q.append(kq_chunk("k", p, 1))
                    fill_q.append(kq_chunk("q", p, 0))
                for p in range(NP):
                    fill_q.append(kq_chunk("q", p, 1))

                # flat slot stream; PV stage lags 2 slots behind scores so
                # ACT never drains at pair boundaries
                slots = [(p, 0, jp) for p in range(NP) for jp in range(4)] + \
                        [(p, 1, jp) for p in range(NP) for jp in range(4)]
                seq = [(p, 0) for p in range(NP)] + [(p, 1) for p in range(NP)]
                pend = []
                for i, (p, nbq, jp) in enumerate(slots):
                    if jp == 0:
                        u = seq.index((p, nbq))
                        if u + 1 < len(seq):
                            bias_load(*seq[u + 1])
                    if nbq == 1 and jp == 2 and p in (0, 1, 2):
                        # only after pair-5-nb0's delayed tail (slot 25) has
                        # evacuated atb[:, 5, 0:F]
                        for cot in (2 * p, 2 * p + 1):
                            fill_q.append(proj_chunk(cot, 0))
                    work = scores_emit(p, nbq, jp)
                    if len(pend) >= 2:
                        pv_emit(pend.pop(0))
                    pend.append(work)
                    if i == 0:
                        cp_state[1] = True
                    fill(2 if i < 28 else 1)
                while pend:
                    pv_emit(pend.pop(0))
                fill(len(fill_q))
                # tail: projection for q-half 1
                for cot in range(CT):
                    proj_chunk(cot, 1, tail=True)()

    nc.compile()
    return nc


def _get_nc():
    if "nc" not in _cache:
        _cache["nc"] = _build()
    return _cache["nc"]


def prep_in_maps(x, attn_bias, Wq, Wk, Wv, Wp, bp):
    """Host-side sharding + layout prep (transposes/casts only)."""
    wqT = np.ascontiguousarray(Wq.T).astype(BF16)
    wkT = np.ascontiguousarray(Wk.T).astype(BF16)
    wvT = np.ascontiguousarray(Wv.T).astype(BF16)
    wpT = np.ascontiguousarray(Wp.T).astype(BF16)
    bpT = np.ascontiguousarray(bp.astype(np.float32).reshape(CT, 128).T)
    expbT = np.ascontiguousarray(
        np.exp(attn_bias[0].astype(np.float32)).transpose(0, 2, 1)
    ).astype(BF16)
    in_maps = []
    for b in range(B):
        in_maps.append({
            "xT": np.ascontiguousarray(x[b].T).astype(BF16),
            "wqT": wqT, "wkT": wkT, "wvT": wvT, "wpT": wpT,
            "bpT": bpT, "expbT": expbT,
        })
    return in_maps


def run(in_maps, trace=False, **kw):
    from concourse.bass_utils import run_bass_kernel_spmd

    nc = _get_nc()
    return run_bass_kernel_spmd(
        nc, in_maps, core_ids=list(range(B)), trace=trace, **kw
    )


def kernel(x, attn_bias, Wq, Wk, Wv, Wp, bp):
    res = run(prep_in_maps(x, attn_bias, Wq, Wk, Wv, Wp, bp))
    out = np.stack(
        [res.results[b]["outT"].T for b in range(B)]
    ).astype(np.float32)
    return out


# revision 13
# speedup vs baseline: 1.4759x; 1.1547x over previous
"""Multi-head attention (B=8, N=1024, C=768, H=12, D=64) on 8 TRN2 NeuronCores.

Strategy: data-parallel over batch (B == n_cores == 8), no collectives.
v2 design, fully transposed layout (channels on SBUF partitions):

  - Scores: heads processed in pairs (h0=2p on partitions 0:64, h1=2p+1 on
    64:128).  The two heads' score matmuls (contraction K=D=64) are emitted
    back-to-back with row tile_positions (0,0)/(64,0), so the PE runs them
    concurrently in separate array halves -> ~2x score throughput.
  - Score PSUM tiles are [128, 1024] (two banks: key-tiles 2jp, 2jp+1), so a
    single ACT exp instruction covers 1024 elements, amortizing overhead.
  - Bias handled as P = exp(0.125*S^T) * exp(bias)^T: exp(bias) precomputed
    host-side (bf16), applied on DVE in 2x-rate 16-bit mode (cheaper than the
    f32 PSUM add at 1x).
  - PV keeps the ones-row trick (lhsT = [V_h | 1], M=65) for softmax sums.
  - Normalization per (pair, nb-half): DVE reciprocal of the PSUM sum row,
    DMA-broadcast across partitions, fused into the PV evacuation multiply.
  - Loop order: nb (query half) outer, head-pair inner.  Bias DMA is split
    per (head, nb) so each byte is loaded once.  Output projection for
    q-half 0 runs as PE filler during q-half 1's attention.
  - QKV projections + output projection are emitted as "filler" chunks
    inside the ACT-bound attention loop to keep the PE dense (HAM warm).
"""

import os
import sys
import numpy as np

for _p in ("/opt/trn_rl_repo", "/root/.axon_site/_ro/trn_rl_repo"):
    if os.path.isdir(_p) and _p not in sys.path:
        sys.path.append(_p)

import ml_dtypes

BF16 = ml_dtypes.bfloat16

B, N, C = 8, 1024, 768
H, D = 12, 64
CT = C // 128        # 6 channel tiles
NT = N // 128        # 8 key tiles
F = 512
NP = H // 2          # 6 head pairs

_cache = {}


def _build():
    import concourse.bass as bass
    import concourse.tile as tile
    from concourse import bacc, mybir

    f32 = mybir.dt.float32
    bf16 = mybir.dt.bfloat16
    AF = mybir.ActivationFunctionType
    ALU = mybir.AluOpType

    nc = bacc.Bacc("TRN2", target_bir_lowering=False)

    xT_d = nc.dram_tensor("xT", [C, N], bf16, kind="ExternalInput")
    wqT_d = nc.dram_tensor("wqT", [C, C], bf16, kind="ExternalInput")
    wkT_d = nc.dram_tensor("wkT", [C, C], bf16, kind="ExternalInput")
    wvT_d = nc.dram_tensor("wvT", [C, C], bf16, kind="ExternalInput")
    wpT_d = nc.dram_tensor("wpT", [C, C], bf16, kind="ExternalInput")
    bpT_d = nc.dram_tensor("bpT", [128, CT], f32, kind="ExternalInput")
    expbT_d = nc.dram_tensor("expbT", [H, N, N], bf16, kind="ExternalInput")
    outT_d = nc.dram_tensor("outT", [C, N], f32, kind="ExternalOutput")
    rscr_d = nc.dram_tensor("rscr", [H, 2, F], f32)  # reciprocal bounce

    with tile.TileContext(nc) as tc:
        with tc.tile_pool(name="persist", bufs=1) as pers:
            xTb = pers.tile([128, CT, N], bf16, tag="xT")
            wqb = pers.tile([128, CT, C], bf16, tag="wq")
            wkb = pers.tile([128, CT, C], bf16, tag="wk")
            wvb = pers.tile([128, CT, C], bf16, tag="wv")
            wpb = pers.tile([128, CT, C], bf16, tag="wp")
            bpb = pers.tile([128, CT], f32, tag="bp")
            qtb = pers.tile([128, CT, N], bf16, tag="qt")
            ktb = pers.tile([128, CT, N], bf16, tag="kt")
            vb = pers.tile([128, NT, H, D + 1], bf16, tag="v")
            atb = pers.tile([128, CT, N], bf16, tag="at")
            dum = pers.tile([1, 8], f32, tag="dum")

            # ---- phase 0: input DMAs (pair-0 essentials first) -----------
            for i, q0 in enumerate(range(0, N, 256)):
                eng = nc.sync if i % 2 == 0 else nc.scalar
                eng.dma_start(
                    xTb[:, :, q0:q0 + 256],
                    xT_d[:, q0:q0 + 256].rearrange("(ci p) n -> p ci n", p=128))
            nc.scalar.dma_start(
                wvb[:, :, 0:128],
                wvT_d[:, 0:128].rearrange("(ci p) o -> p ci o", p=128))
            nc.sync.dma_start(
                wkb[:, :, 0:128],
                wkT_d[:, 0:128].rearrange("(ci p) o -> p ci o", p=128))
            nc.scalar.dma_start(
                wqb[:, :, 0:128],
                wqT_d[:, 0:128].rearrange("(ci p) o -> p ci o", p=128))
            nc.scalar.dma_start(bpb, bpT_d[:])
            nc.sync.dma_start(
                wvb[:, :, 128:C],
                wvT_d[:, 128:C].rearrange("(ci p) o -> p ci o", p=128))
            nc.scalar.dma_start(
                wkb[:, :, 128:C],
                wkT_d[:, 128:C].rearrange("(ci p) o -> p ci o", p=128))
            nc.sync.dma_start(
                wqb[:, :, 128:C],
                wqT_d[:, 128:C].rearrange("(ci p) o -> p ci o", p=128))
            nc.scalar.dma_start(
                wpb[:, :, 0:384],
                wpT_d[:, 0:384].rearrange("(ci p) o -> p ci o", p=128))
            nc.sync.dma_start(
                wpb[:, :, 384:C],
                wpT_d[:, 384:C].rearrange("(ci p) o -> p ci o", p=128))

            nc.vector.memset(vb[:, :, :, D:D + 1], 1.0)
            nc.vector.memset(dum, 1.0)
            nc.scalar.activation(dum, dum, AF.Exp)  # preload exp table set

            with tc.tile_pool(name="sps", bufs=2, space="PSUM") as sp, \
                 tc.tile_pool(name="pvps", bufs=2, space="PSUM") as pvp, \
                 tc.tile_pool(name="ups", bufs=2, space="PSUM") as ups, \
                 tc.tile_pool(name="ptp", bufs=6) as ptp, \
                 tc.tile_pool(name="btp", bufs=4) as btp, \
                 tc.tile_pool(name="rbp", bufs=4) as rbp, \
                 tc.tile_pool(name="rrp", bufs=4) as rrp, \
                 tc.tile_pool(name="vstp", bufs=2) as vstp, \
                 tc.tile_pool(name="otp", bufs=3) as otp:

                cp_state = [0, False]  # [rotation, attention_started]

                def cp(dst, src):
                    """PSUM->SBUF copy.  DVE-only once attention starts
                    (an ACT-queued copy behind EXPs stalls ups recycling)."""
                    if cp_state[1]:
                        nc.vector.tensor_copy(dst, src)
                        return
                    cp_state[0] = (cp_state[0] + 1) % 3
                    if cp_state[0]:
                        nc.vector.tensor_copy(dst, src)
                    else:
                        nc.scalar.copy(dst, src)

                def v_chunk(p, nts):
                    """V projection for heads 2p, 2p+1 over key tiles nts."""
                    f0 = p * 128

                    def go():
                        for nt in nts:
                            ps = ups.tile([128, F], f32, tag="u")
                            for ci in range(CT):
                                nc.tensor.matmul(
                                    ps[:, 0:128],
                                    lhsT=xTb[:, ci, nt * 128:(nt + 1) * 128],
                                    rhs=wvb[:, ci, f0:f0 + 128],
                                    start=(ci == 0),
                                    stop=(ci == CT - 1),
                                )
                            cp(vb[:, nt, 2 * p:2 * p + 2, 0:D],
                               ps[:, 0:128].rearrange("p (h d) -> p h d", d=D))
                    return go

                def kq_chunk(which, cot, nbq):
                    wb, dst = (wkb, ktb) if which == "k" else (wqb, qtb)

                    def go():
                        ps = ups.tile([128, F], f32, tag="u")
                        for ci in range(CT):
                            nc.tensor.matmul(
                                ps,
                                lhsT=wb[:, ci, cot * 128:(cot + 1) * 128],
                                rhs=xTb[:, ci, nbq * F:(nbq + 1) * F],
                                start=(ci == 0),
                                stop=(ci == CT - 1),
                            )
                        cp(dst[:, cot, nbq * F:(nbq + 1) * F], ps)
                    return go

                def proj_chunk(cot, nbq, tail=False):
                    def go():
                        ps = ups.tile([128, F], f32, tag="u")
                        for ci in range(CT):
                            nc.tensor.matmul(
                                ps,
                                lhsT=wpb[:, ci, cot * 128:(cot + 1) * 128],
                                rhs=atb[:, ci, nbq * F:(nbq + 1) * F],
                                start=(ci == 0),
                                stop=(ci == CT - 1),
                            )
                        ot = otp.tile([128, F], f32, tag="ot")
                        if tail:
                            nc.scalar.activation(
                                ot, ps, AF.Identity, bias=bpb[:, cot:cot + 1])
                        else:
                            nc.vector.tensor_scalar_add(
                                ot, ps, bpb[:, cot:cot + 1])
                        eng = nc.sync if cot % 2 else nc.scalar
                        eng.dma_start(
                            outT_d[cot * 128:(cot + 1) * 128,
                                   nbq * F:(nbq + 1) * F],
                            ot,
                        )
                    return go

                fill_q = []

                def fill(n):
                    for _ in range(n):
                        if fill_q:
                            fill_q.pop(0)()

                bt_map = {}

                def bias_load(p, nbq):
                    for h in (2 * p, 2 * p + 1):
                        bt = btp.tile([128, NT, F], bf16, tag="bt")
                        src = expbT_d[h].rearrange("(j p) q -> p j q", p=128)
                        nc.sync.dma_start(
                            bt[:, 0:NT // 2, :],
                            src[:, 0:NT // 2, nbq * F:(nbq + 1) * F])
                        nc.sync.dma_start(
                            bt[:, NT // 2:NT, :],
                            src[:, NT // 2:NT, nbq * F:(nbq + 1) * F])
                        bt_map[(h, nbq)] = bt

                pair_state = {}

                def scores_emit(p, nbq, jp):
                    """Score matmuls + exp + bias-mult for one jp slot."""
                    h0, h1 = 2 * p, 2 * p + 1
                    qsl = slice(nbq * F, (nbq + 1) * F)
                    if jp == 0:
                        pair_state[(p, nbq)] = {
                            "bt0": bt_map.pop((h0, nbq)),
                            "bt1": bt_map.pop((h1, nbq)),
                        }
                    st = pair_state[(p, nbq)]
                    j0, j1 = 2 * jp, 2 * jp + 1
                    s0 = sp.tile([128, 2 * F], f32, tag="s", name=f"s0_{jp}")
                    s1 = sp.tile([128, 2 * F], f32, tag="s", name=f"s1_{jp}")
                    # row-tile pairs: (0,0) and (64,0) run concurrently
                    nc.tensor.matmul(
                        s0[:, 0:F],
                        lhsT=ktb[0:64, p, j0 * 128:(j0 + 1) * 128],
                        rhs=qtb[0:64, p, qsl], start=True, stop=True)
                    nc.tensor.matmul(
                        s1[:, 0:F],
                        lhsT=ktb[64:128, p, j0 * 128:(j0 + 1) * 128],
                        rhs=qtb[64:128, p, qsl], start=True, stop=True)
                    nc.tensor.matmul(
                        s0[:, F:2 * F],
                        lhsT=ktb[0:64, p, j1 * 128:(j1 + 1) * 128],
                        rhs=qtb[0:64, p, qsl], start=True, stop=True)
                    nc.tensor.matmul(
                        s1[:, F:2 * F],
                        lhsT=ktb[64:128, p, j1 * 128:(j1 + 1) * 128],
                        rhs=qtb[64:128, p, qsl], start=True, stop=True)
                    pt0 = ptp.tile([128, 2 * F], bf16, tag="pt", name=f"pt0_{jp}")
                    pt1 = ptp.tile([128, 2 * F], bf16, tag="pt", name=f"pt1_{jp}")
                    nc.scalar.activation(pt0, s0, AF.Exp, scale=0.125)
                    nc.vector.tensor_tensor(
                        pt0.rearrange("p (j q) -> p j q", j=2),
                        pt0.rearrange("p (j q) -> p j q", j=2),
                        st["bt0"][:, j0:j0 + 2, :], ALU.mult)
                    nc.scalar.activation(pt1, s1, AF.Exp, scale=0.125)
                    nc.vector.tensor_tensor(
                        pt1.rearrange("p (j q) -> p j q", j=2),
                        pt1.rearrange("p (j q) -> p j q", j=2),
                        st["bt1"][:, j0:j0 + 2, :], ALU.mult)
                    return (p, nbq, jp, pt0, pt1)

                def pv_emit(work):
                    p, nbq, jp, pt0, pt1 = work
                    h0, h1 = 2 * p, 2 * p + 1
                    st = pair_state[(p, nbq)]
                    if jp == 0:
                        # allocate accumulators here: by program order the
                        # previous pair's last access to these PSUM banks
                        # (its evac) has already been emitted
                        st["pv0"] = pvp.tile([D + 1, F], f32, tag="pv",
                                             name=f"pv0_{p}_{nbq}")
                        st["pv1"] = pvp.tile([D + 1, F], f32, tag="pv",
                                             name=f"pv1_{p}_{nbq}")
                    j0, j1 = 2 * jp, 2 * jp + 1
                    nc.tensor.matmul(
                        st["pv0"], lhsT=vb[:, j0, h0, :], rhs=pt0[:, 0:F],
                        start=(jp == 0), stop=False)
                    nc.tensor.matmul(
                        st["pv1"], lhsT=vb[:, j0, h1, :], rhs=pt1[:, 0:F],
                        start=(jp == 0), stop=False)
                    nc.tensor.matmul(
                        st["pv0"], lhsT=vb[:, j1, h0, :], rhs=pt0[:, F:2 * F],
                        start=False, stop=(jp == 3))
                    nc.tensor.matmul(
                        st["pv1"], lhsT=vb[:, j1, h1, :], rhs=pt1[:, F:2 * F],
                        start=False, stop=(jp == 3))
                    if jp == 3:
                        pair_tail(p, nbq, last=(p == NP - 1 and nbq == 1))

                def pair_tail(p, nbq, last=False):
                    """Evacuate unnormalized (frees PSUM fast), then
                    reciprocal + in-place normalization off the critical
                    path (GpSimd; DVE for the final unit's tail)."""
                    h0, h1 = 2 * p, 2 * p + 1
                    qsl = slice(nbq * F, (nbq + 1) * F)
                    st = pair_state.pop((p, nbq))
                    pv0, pv1 = st["pv0"], st["pv1"]
                    sr0 = rrp.tile([1, F], f32, tag="sr")
                    nc.vector.tensor_copy(sr0, pv0[D:D + 1, :])
                    nc.vector.tensor_copy(atb[0:64, p, qsl], pv0[0:D, :])
                    sr1 = rrp.tile([1, F], f32, tag="sr")
                    nc.vector.tensor_copy(sr1, pv1[D:D + 1, :])
                    vst = vstp.tile([D, F], bf16, tag="vst")
                    nc.vector.tensor_copy(vst, pv1[0:D, :])
                    nc.gpsimd.dma_start(atb[64:128, p, qsl], vst)
                    rr0 = rrp.tile([1, F], f32, tag="rr")
                    nc.vector.reciprocal_approx_fast(rr0, sr0)
                    nc.gpsimd.dma_start(rscr_d[h0, nbq:nbq + 1, :], rr0)
                    rb0 = rbp.tile([128, F], f32, tag="rb")
                    nc.sync.dma_start(
                        rb0, rscr_d[h0, nbq:nbq + 1, :].to_broadcast([128, F]))
                    rr1 = rrp.tile([1, F], f32, tag="rr")
                    nc.vector.reciprocal_approx_fast(rr1, sr1)
                    nc.gpsimd.dma_start(rscr_d[h1, nbq:nbq + 1, :], rr1)
                    rb1 = rbp.tile([128, F], f32, tag="rb")
                    nc.sync.dma_start(
                        rb1, rscr_d[h1, nbq:nbq + 1, :].to_broadcast([128, F]))
                    eng = nc.vector if last else nc.gpsimd
                    eng.tensor_tensor(
                        atb[0:64, p, qsl], atb[0:64, p, qsl],
                        rb0[0:64, :], ALU.mult)
                    eng.tensor_tensor(
                        atb[64:128, p, qsl], atb[64:128, p, qsl],
                        rb1[64:128, :], ALU.mult)

                # ---- emission schedule --------------------------------
                # units interleave nb halves per pair so PE filler (V/K/Q)
                # spreads evenly across all slots; p5-nb0 hoisted before
                # p4-nb1 so proj(nb0) still hides under the last two units
                seq = [(0, 0), (0, 1), (1, 0), (1, 1), (2, 0), (2, 1),
                       (3, 0), (3, 1), (4, 0), (5, 0), (4, 1), (5, 1)]
                bias_load(0, 0)
                # pair-0 essentials, emitted directly
                v_chunk(0, range(0, NT))()
                kq_chunk("k", 0, 0)()
                kq_chunk("k", 0, 1)()
                kq_chunk("q", 0, 0)()
                kq_chunk("q", 0, 1)()

                for p in range(1, NP):
                    fill_q.append(v_chunk(p, range(0, NT // 2)))
                    fill_q.append(v_chunk(p, range(NT // 2, NT)))
                    fill_q.append(kq_chunk("k", p, 0))
                    fill_q.append(kq_chunk("k", p, 1))
                    fill_q.append(kq_chunk("q", p, 0))
                    fill_q.append(kq_chunk("q", p, 1))

                slots = [(p, nbq, jp) for (p, nbq) in seq for jp in range(4)]
                pend = []
                for i, (p, nbq, jp) in enumerate(slots):
                    if jp == 0:
                        u = seq.index((p, nbq))
                        if u + 1 < len(seq):
                            bias_load(*seq[u + 1])
                    if (p, nbq, jp) == (4, 1, 2):
                        # after (5,0)'s delayed tail has evacuated atb nb0
                        for cot in range(CT):
                            fill_q.append(proj_chunk(cot, 0))
                    work = scores_emit(p, nbq, jp)
                    if len(pend) >= 2:
                        pv_emit(pend.pop(0))
                    pend.append(work)
                    if i == 0:
                        cp_state[1] = True
                    fill(1)
                while pend:
                    pv_emit(pend.pop(0))
                fill(len(fill_q))
                # tail: projection for q-half 1
                for cot in range(CT):
                    proj_chunk(cot, 1, tail=True)()

    nc.compile()
    return nc


def _get_nc():
    if "nc" not in _cache:
        _cache["nc"] = _build()
    return _cache["nc"]


def prep_in_maps(x, attn_bias, Wq, Wk, Wv, Wp, bp):
    """Host-side sharding + layout prep (transposes/casts only)."""
    wqT = np.ascontiguousarray(Wq.T).astype(BF16)
    wkT = np.ascontiguousarray(Wk.T).astype(BF16)
    wvT = np.ascontiguousarray(Wv.T).astype(BF16)
    wpT = np.ascontiguousarray(Wp.T).astype(BF16)
    bpT = np.ascontiguousarray(bp.astype(np.float32).reshape(CT, 128).T)
    expbT = np.ascontiguousarray(
        np.exp(attn_bias[0].astype(np.float32)).transpose(0, 2, 1)
    ).astype(BF16)
    in_maps = []
    for b in range(B):
        in_maps.append({
            "xT": np.ascontiguousarray(x[b].T).astype(BF16),
            "wqT": wqT, "wkT": wkT, "wvT": wvT, "wpT": wpT,
            "bpT": bpT, "expbT": expbT,
        })
    return in_maps


def run(in_maps, trace=False, **kw):
    from concourse.bass_utils import run_bass_kernel_spmd

    nc = _get_nc()
    return run_bass_kernel_spmd(
        nc, in_maps, core_ids=list(range(B)), trace=trace, **kw
    )


def kernel(x, attn_bias, Wq, Wk, Wv, Wp, bp):
    res = run(prep_in_maps(x, attn_bias, Wq, Wk, Wv, Wp, bp))
    out = np.stack(
        [res.results[b]["outT"].T for b in range(B)]
    ).astype(np.float32)
    return out
